# revision 1
# baseline (speedup 1.0000x reference)
"""GPT forward pass on 8 Trainium2 NeuronCores.

Sharding: cores 0-3 handle batch 0, cores 4-7 batch 1; within each 4-core
group the 1024 tokens are sequence-sharded 256/core. Activations are kept
feature-major (transposed) on chip so no on-device transposes are needed.
Per layer each core computes Q/K/V for its own tokens; K then V are
AllGathered (bf16, two pipelined collectives) within the 4-core group
(layer 0 computes K/V from the full h0 directly, no collective). The final
vocab projection is token-sharded: each core projects its own 256 tokens
against the full 32000-dim vocab with h-stationary matmuls (no final
AllGather); the output bias is added on the host.
"""

import os
import sys

for _p in ("/opt/trn_rl_repo", "/root/.axon_site/_ro/trn_rl_repo"):
    if os.path.isdir(_p) and _p not in sys.path:
        sys.path.insert(0, _p)

import ml_dtypes
import numpy as np

import concourse.bass as bass
import concourse.mybir as mybir
import concourse.tile as tile
from concourse import bacc
from concourse.bass_utils import run_bass_kernel_spmd

BF16 = ml_dtypes.bfloat16
f32 = mybir.dt.float32
bf16 = mybir.dt.bfloat16
AF = mybir.ActivationFunctionType
ALU = mybir.AluOpType

V, S, E, H, D, L = 32000, 1024, 512, 8, 64, 4
FF = 4 * E
B = 2
NC = 8
G = 4            # cores per batch group
TO = S // G      # tokens owned per core (256)
EPS = 1e-5
NKT = S // 128   # key tiles (8)
NFT = E // 128   # feature tiles (4)
NTT = TO // 128  # own-token tiles (2)
NMT = FF // 128  # FFN hidden tiles (16)
NV = 500         # vocab columns per projection chunk
NVC = V // NV    # 64 projection chunks

_cache = {}
COLLFREE = False

def build_nc(trace=False, rep=1):
    nc = bacc.Bacc("TRN2", target_bir_lowering=False, debug=False,
                   num_devices=1 if COLLFREE else NC)

    def din(name, shape, dt):
        return nc.dram_tensor(name, shape, dt, kind="ExternalInput").ap()

    io = dict(
        h0t_full=din("h0t_full", [E, S], bf16),
        h0t_own=din("h0t_own", [E, TO], f32),
        maskF=din("maskF", [S, TO], bf16),
        maskB=din("maskB", [128, NKT], f32),
        maskOwn=din("maskOwn", [2 * 128, TO], bf16),
        wq=din("wq", [L, E, H * D], bf16),
        wk=din("wk", [L, E, H * D], bf16),
        wv=din("wv", [L, E, H * D], bf16),
        wo=din("wo", [L, H * D, E], bf16),
        w1=din("w1", [L, E, FF], bf16),
        w2=din("w2", [L, FF, E], bf16),
        bq=din("bq", [L, E], f32),
        bk=din("bk", [L, E], f32),
        bo=din("bo", [L, E], f32),
        b1=din("b1", [L, FF], f32),
        b2=din("b2", [L, E], f32),
        g1=din("g1", [L, E], f32),
        be1=din("be1", [L, E], f32),
        g2=din("g2", [L, E], f32),
        be2=din("be2", [L, E], f32),
        wout=din("wout", [NVC, 128, NFT * NV], bf16),
        out=nc.dram_tensor("out", [NTT, NVC, 128, NV], bf16,
                           kind="ExternalOutput").ap(),
    )

    with tile.TileContext(nc) as tc:
        _body(nc, tc, io, rep=rep)
    nc.compile()
    return nc


class P:
    """pool handles"""


def _body(nc, tc, io, rep=1):
    from contextlib import ExitStack

    ctx = ExitStack()
    with ctx:
        p = P()
        p.w512 = ctx.enter_context(tc.tile_pool(name="w512", bufs=5))
        p.wff = ctx.enter_context(tc.tile_pool(name="wff", bufs=3))
        p.wout = ctx.enter_context(tc.tile_pool(name="pwout", bufs=6))
        p.kv = ctx.enter_context(tc.tile_pool(name="kv", bufs=1))
        p.act = ctx.enter_context(tc.tile_pool(name="act", bufs=1))
        p.a1 = ctx.enter_context(tc.tile_pool(name="a1p", bufs=NMT + 2))
        p.exp = ctx.enter_context(tc.tile_pool(name="exp", bufs=3))
        p.const = ctx.enter_context(tc.tile_pool(name="const", bufs=1))
        p.stat = ctx.enter_context(tc.tile_pool(name="stat", bufs=2))
        p.out = ctx.enter_context(tc.tile_pool(name="pout", bufs=3))
        p.mm = ctx.enter_context(tc.tile_pool(name="pmm", bufs=3, space="PSUM"))
        p.ua = ctx.enter_context(tc.tile_pool(name="uaug", bufs=4, space="PSUM"))
        p.misc = ctx.enter_context(tc.tile_pool(name="psmisc", bufs=1, space="PSUM"))
        p.dram = ctx.enter_context(tc.tile_pool(name="dram", bufs=2, space="DRAM"))

        def mmtile():
            return p.mm.tile([128, 512], f32, tag="mm", name="mm")

        # ---- constants ----
        ones_f = p.const.tile([128, 1], f32, tag="ones_f")
        nc.vector.memset(ones_f[:], 1.0)
        ones_r = p.const.tile([1, 128], f32, tag="ones_r")
        nc.vector.memset(ones_r[:], 1.0)
        zeros_r = p.const.tile([1, 512], bf16, tag="zeros_r")
        nc.vector.memset(zeros_r[:], 0.0)

        def ldvec(ap, name, n=NFT):
            t = p.const.tile([128, L * n], f32, tag=name)
            nc.sync.dma_start(t[:], ap.rearrange("l (k p) -> p (l k)", p=128)[:])
            return t

        bq_t = ldvec(io["bq"], "bq")
        bk_t = ldvec(io["bk"], "bk")
        bo_t = ldvec(io["bo"], "bo")
        b2_t = ldvec(io["b2"], "b2")
        g1_t = ldvec(io["g1"], "g1")
        be1_t = ldvec(io["be1"], "be1")
        g2_t = ldvec(io["g2"], "g2")
        be2_t = ldvec(io["be2"], "be2")
        b1_t = ldvec(io["b1"], "b1", n=NMT)

        mask_t = []
        for kt in range(NKT):
            m = p.const.tile([128, TO], bf16, tag=f"mask{kt}")
            nc.sync.dma_start(m[:], io["maskF"][kt * 128:(kt + 1) * 128, :])
            mask_t.append(m)
        maskb_t = p.const.tile([128, NKT], f32, tag="maskb")
        nc.sync.dma_start(maskb_t[:], io["maskB"][:, :])
        maskown_t = []
        for lt in range(NTT):
            mo = p.const.tile([128, TO], bf16, tag=f"masko{lt}")
            nc.sync.dma_start(mo[:], io["maskOwn"][lt * 128:(lt + 1) * 128, :])
            maskown_t.append(mo)

        # ---- initial hidden state ----
        ht = []
        for kf in range(NFT):
            t = p.act.tile([128, TO], f32, tag=f"ht{kf}")
            nc.sync.dma_start(t[:], io["h0t_own"][kf * 128:(kf + 1) * 128, :])
            ht.append(t)
        h0f_b = []
        for kf in range(NFT):
            t = p.const.tile([128, S], bf16, tag=f"h0fb{kf}")
            nc.sync.dma_start(t[:], io["h0t_full"][kf * 128:(kf + 1) * 128, :])
            h0f_b.append(t)

        def load_w512(ap, l):
            t = p.w512.tile([128, NFT * 512], bf16, tag="w512")
            nc.sync.dma_start(t[:].rearrange("p (k n) -> p k n", k=NFT),
                              ap[l].rearrange("(k p) n -> p k n", p=128)[:])
            return t

        htb = None
        for _rep in range(rep):
            _compute(nc, tc, io, p, locals())


def _compute(nc, tc, io, p, env):
    mmtile = env["mmtile"]
    ones_f = env["ones_f"]; ones_r = env["ones_r"]
    zeros_r = env["zeros_r"]
    bq_t = env["bq_t"]; bk_t = env["bk_t"]; bo_t = env["bo_t"]; b2_t = env["b2_t"]
    g1_t = env["g1_t"]; be1_t = env["be1_t"]; g2_t = env["g2_t"]; be2_t = env["be2_t"]
    b1_t = env["b1_t"]; mask_t = env["mask_t"]
    maskb_t = env["maskb_t"]; maskown_t = env["maskown_t"]
    ht = env["ht"]; h0f_b = env["h0f_b"]; load_w512 = env["load_w512"]
    if True:
        # ================= transformer layers =================
        for l in range(L):
            wq_t = load_w512(io["wq"], l)
            wk_t = load_w512(io["wk"], l)
            wv_t = load_w512(io["wv"], l)

            hb = []
            for kf in range(NFT):
                b = p.act.tile([128, TO], bf16, tag=f"hb{kf}")
                nc.vector.tensor_copy(b[:], ht[kf][:])
                hb.append(b)

            kt_all = []   # 4 tiles [128, S] bf16: gathered K^T
            vt_all = []   # 8 tiles [128, H*65] bf16: V with ones column per head
            if l == 0:
                for mf in range(NFT):
                    kt_t = p.kv.tile([128, S], bf16, tag=f"kt{mf}")
                    for c2 in range(S // 512):
                        ps = mmtile()
                        for kf in range(NFT):
                            nc.tensor.matmul(
                                ps[:],
                                wk_t[:, kf * 512 + mf * 128: kf * 512 + (mf + 1) * 128],
                                h0f_b[kf][:, c2 * 512:(c2 + 1) * 512],
                                start=(kf == 0), stop=(kf == NFT - 1))
                        nc.vector.tensor_scalar_add(
                            kt_t[:, c2 * 512:(c2 + 1) * 512], ps[:],
                            bk_t[:, l * NFT + mf: l * NFT + mf + 1])
                    kt_all.append(kt_t)
                for tt in range(NKT):
                    vt_t = p.kv.tile([128, H * 65], bf16, tag=f"vt{tt}")
                    ps = mmtile()
                    for kf in range(NFT):
                        nc.tensor.matmul(
                            ps[:],
                            h0f_b[kf][:, tt * 128:(tt + 1) * 128],
                            wv_t[:, kf * 512:(kf + 1) * 512],
                            start=(kf == 0), stop=(kf == NFT - 1))
                    nc.vector.tensor_copy(
                        vt_t.rearrange("p (h e) -> p h e", h=H)[:, :, 0:64],
                        ps.rearrange("p (h e) -> p h e", h=H)[:, :, :])
                    nc.vector.memset(
                        vt_t.rearrange("p (h e) -> p h e", h=H)[:, :, 64:65], 1.0)
                    vt_all.append(vt_t)
            klocal = []
            vtloc = []
            if l > 0:
                # K contribution first -> AllGather K while V projects
                contribK = p.dram.tile([E * TO], bf16, tag="contribK")
                for mf in range(NFT):
                    ps = mmtile()
                    for kf in range(NFT):
                        nc.tensor.matmul(
                            ps[:, :TO],
                            wk_t[:, kf * 512 + mf * 128: kf * 512 + (mf + 1) * 128],
                            hb[kf][:],
                            start=(kf == 0), stop=(kf == NFT - 1))
                    kl = p.act.tile([128, TO], bf16, tag=f"klocal{mf}", bufs=2)
                    nc.vector.tensor_scalar_add(
                        kl[:], ps[:, :TO], bk_t[:, l * NFT + mf: l * NFT + mf + 1])
                    klocal.append(kl)
                    nc.sync.dma_start(
                        contribK.rearrange("(p n) -> p n", p=E)[
                            mf * 128:(mf + 1) * 128, :],
                        kl[:])
                gathK = p.dram.tile([G, E * TO], bf16, tag="gathK")
                if COLLFREE:
                    for rr in range(G):
                        nc.sync.dma_start(gathK[rr], contribK[:])
                else:
                    nc.gpsimd.collective_compute(
                        "AllGather", ALU.bypass,
                        replica_groups=[[0, 1, 2, 3], [4, 5, 6, 7]],
                        ins=[contribK[:]], outs=[gathK[:]])

                contribV = p.dram.tile([TO * E], bf16, tag="contribV")
                for tt in range(NTT):
                    ps = mmtile()
                    for kf in range(NFT):
                        nc.tensor.matmul(
                            ps[:],
                            hb[kf][:, tt * 128:(tt + 1) * 128],
                            wv_t[:, kf * 512:(kf + 1) * 512],
                            start=(kf == 0), stop=(kf == NFT - 1))
                    vl = p.act.tile([128, 512], bf16, tag=f"vlocal{tt}", bufs=2)
                    nc.vector.tensor_copy(vl[:], ps[:])
                    nc.sync.dma_start(
                        contribV.rearrange("(t e) -> t e", e=E)[
                            tt * 128:(tt + 1) * 128, :],
                        vl[:])
                    vt_l = p.act.tile([128, H * 65], bf16, tag=f"vtloc{tt}",
                                      bufs=2)
                    nc.vector.tensor_copy(
                        vt_l.rearrange("p (h e) -> p h e", h=H)[:, :, 0:64],
                        vl[:].rearrange("p (h e) -> p h e", h=H)[:, :, :])
                    nc.vector.memset(
                        vt_l.rearrange("p (h e) -> p h e", h=H)[:, :, 64:65],
                        1.0)
                    vtloc.append(vt_l)
                gathV = p.dram.tile([G, TO * E], bf16, tag="gathV")
                if COLLFREE:
                    for rr in range(G):
                        nc.sync.dma_start(gathV[rr], contribV[:])
                else:
                    nc.gpsimd.collective_compute(
                        "AllGather", ALU.bypass,
                        replica_groups=[[0, 1, 2, 3], [4, 5, 6, 7]],
                        ins=[contribV[:]], outs=[gathV[:]])

                kg = gathK.rearrange("r (p c) -> p r c", p=E)
                vg = gathV.rearrange("r (t e) -> r t e", e=E)
                for mf in range(NFT):
                    kt_t = p.kv.tile([128, S], bf16, tag=f"kt{mf}")
                    nc.sync.dma_start(
                        kt_t[:].rearrange("p (r c) -> p r c", r=G),
                        kg[mf * 128:(mf + 1) * 128, :, :])
                    kt_all.append(kt_t)
                for tt in range(NKT):
                    vt_t = p.kv.tile([128, H * 65], bf16, tag=f"vt{tt}")
                    nc.sync.dma_start(
                        vt_t.rearrange("p (h e) -> p h e", h=H)[:, :, 0:64],
                        vg[tt // 2, (tt % 2) * 128:(tt % 2) * 128 + 128, :].rearrange(
                            "t (h e) -> t h e", h=H)[:])
                    nc.vector.memset(
                        vt_t.rearrange("p (h e) -> p h e", h=H)[:, :, 64:65], 1.0)
                    vt_all.append(vt_t)

            # Q^T [E, TO] bf16 (1/sqrt(D) folded into wq/bq on host)
            qt = []
            for mf in range(NFT):
                ps = mmtile()
                for kf in range(NFT):
                    nc.tensor.matmul(
                        ps[:, :TO],
                        wq_t[:, kf * 512 + mf * 128: kf * 512 + (mf + 1) * 128],
                        hb[kf][:],
                        start=(kf == 0), stop=(kf == NFT - 1))
                q = p.act.tile([128, TO], bf16, tag=f"qt{mf}")
                nc.vector.tensor_scalar_add(
                    q[:], ps[:, :TO], bq_t[:, l * NFT + mf: l * NFT + mf + 1])
                qt.append(q)

            wo_t = load_w512(io["wo"], l)
            w1a = p.wff.tile([128, 4096], bf16, tag="wff")
            nc.sync.dma_start(
                w1a[:].rearrange("p (k n) -> p k n", k=4),
                io["w1"][l][:, 0:1024].rearrange("(k p) n -> p k n", p=128)[:])
            w1b = p.wff.tile([128, 4096], bf16, tag="wff")
            nc.sync.dma_start(
                w1b[:].rearrange("p (k n) -> p k n", k=4),
                io["w1"][l][:, 1024:2048].rearrange("(k p) n -> p k n", p=128)[:])

            # ---- attention ----
            # scores for a head pair share one PSUM bank -> single exp per pair
            upair = [p.ua.tile([65, 512], f32, tag="uaug", name=f"ua{i}") for i in range(4)]
            for i in range(4):
                # open the bank's accumulation group across both heads
                nc.tensor.matmul(upair[i][:, :], zeros_r[0:1, 0:65],
                                 zeros_r[0:1, :], start=True, stop=False)
            if l > 0:
                # local pass: own K/V blocks straight from SBUF, overlaps
                # the AllGathers (the gathered-path mask zeroes these rows)
                for lt in range(NTT):
                    esl = p.exp.tile([128, H * TO], bf16, tag="expL", bufs=2)
                    for h in range(H):
                        sp = mmtile()
                        nc.tensor.matmul(
                            sp[:, :TO],
                            klocal[h // 2][64 * (h % 2):64 * (h % 2) + 64,
                                           lt * 128:(lt + 1) * 128],
                            qt[h // 2][64 * (h % 2):64 * (h % 2) + 64, :],
                            start=True, stop=True)
                        nc.scalar.activation(
                            esl[:, h * TO:(h + 1) * TO], sp[:, :TO], AF.Exp)
                        nc.vector.tensor_mul(
                            esl[:, h * TO:(h + 1) * TO],
                            esl[:, h * TO:(h + 1) * TO], maskown_t[lt][:])
                    for h in range(H):
                        nc.tensor.matmul(
                            upair[h // 2][:, 256 * (h % 2):256 * (h % 2) + 256],
                            vtloc[lt][:, h * 65:(h + 1) * 65],
                            esl[:, h * TO:(h + 1) * TO],
                            start=False, stop=False,
                            skip_group_check=True)
            for kt in range(NKT):
                es = p.exp.tile([128, H * TO], bf16, tag="expS")
                for h in range(H):
                    if h == 3 or h == 7:
                        sp = p.misc.tile([128, 512], f32, tag="psmisc",
                                         name=f"spm{l}_{kt}_{h}")
                    else:
                        sp = mmtile()
                    nc.tensor.matmul(
                        sp[:, :TO],
                        kt_all[h // 2][64 * (h % 2):64 * (h % 2) + 64,
                                       kt * 128:(kt + 1) * 128],
                        qt[h // 2][64 * (h % 2):64 * (h % 2) + 64, :],
                        start=True, stop=True)
                    if l == 0:
                        # per-element causal mask (diagonal blocks included)
                        nc.scalar.activation(
                            es[:, h * TO:(h + 1) * TO], sp[:, :TO], AF.Exp)
                        nc.vector.tensor_mul(
                            es[:, h * TO:(h + 1) * TO],
                            es[:, h * TO:(h + 1) * TO], mask_t[kt][:])
                    else:
                        # blocks are all-visible or all-masked per core:
                        # fold the mask into the exp as a -30 bias
                        nc.scalar.activation(
                            es[:, h * TO:(h + 1) * TO], sp[:, :TO], AF.Exp,
                            bias=maskb_t[:, kt:kt + 1])
                for h in range(H):
                    nc.tensor.matmul(
                        upair[h // 2][:, 256 * (h % 2):256 * (h % 2) + 256],
                        vt_all[kt][:, h * 65:(h + 1) * 65],
                        es[:, h * TO:(h + 1) * TO],
                        start=False,
                        stop=(kt == NKT - 1 and h % 2 == 1),
                        skip_group_check=True)

            # normalize heads -> conc^T [E, TO] bf16
            conc = []
            for mf in range(NFT):
                conc.append(p.act.tile([128, TO], bf16, tag=f"conc{mf}", name=f"conc{mf}"))
            for h in range(H):
                rec = p.stat.tile([1, TO], f32, tag="rec")
                nc.vector.reciprocal(
                    rec[:], upair[h // 2][64:65, 256 * (h % 2):256 * (h % 2) + 256])
                rb = p.misc.tile([64, TO], f32, tag="psmisc")
                nc.tensor.matmul(rb[:], ones_r[0:1, 0:64], rec[:],
                                 start=True, stop=True)
                rbs = p.stat.tile([64, TO], f32, tag="rbs")
                nc.vector.tensor_copy(rbs[:], rb[:])
                nc.vector.tensor_mul(
                    conc[h // 2][64 * (h % 2):64 * (h % 2) + 64, :],
                    upair[h // 2][0:64, 256 * (h % 2):256 * (h % 2) + 256],
                    rbs[:])

            # ---- mha^T + residual + LN1 ----
            res1 = []
            for mf in range(NFT):
                ps = mmtile()
                for kf in range(NFT):
                    nc.tensor.matmul(
                        ps[:, :TO],
                        wo_t[:, kf * 512 + mf * 128: kf * 512 + (mf + 1) * 128],
                        conc[kf][:],
                        start=(kf == 0), stop=(kf == NFT - 1))
                r = p.act.tile([128, TO], f32, tag=f"res1{mf}")
                nc.vector.tensor_scalar_add(
                    r[:], ps[:, :TO], bo_t[:, l * NFT + mf: l * NFT + mf + 1])
                nc.vector.tensor_add(r[:], r[:], ht[mf][:])
                res1.append(r)

            ln1f, ln1b = _layernorm(nc, p, ones_f, ones_r, res1,
                                    g1_t, be1_t, l, "ln1", mmtile)

            # ---- FFN ----
            w2a = p.wff.tile([128, 4096], bf16, tag="wff")
            nc.sync.dma_start(
                w2a[:].rearrange("p (k n) -> p k n", k=8),
                io["w2"][l][0:1024, :].rearrange("(k p) n -> p k n", p=128)[:])
            w2b = p.wff.tile([128, 4096], bf16, tag="wff")
            nc.sync.dma_start(
                w2b[:].rearrange("p (k n) -> p k n", k=8),
                io["w2"][l][1024:2048, :].rearrange("(k p) n -> p k n", p=128)[:])

            a1 = []
            for mt in range(NMT):
                wsrc = w1a if mt < 8 else w1b
                moff = mt % 8
                ps = mmtile()
                for kf in range(NFT):
                    nc.tensor.matmul(
                        ps[:, :TO],
                        wsrc[:, kf * 1024 + moff * 128: kf * 1024 + (moff + 1) * 128],
                        ln1b[kf][:],
                        start=(kf == 0), stop=(kf == NFT - 1))
                a = p.a1.tile([128, TO], bf16, tag="a1")
                nc.scalar.activation(
                    a[:], ps[:, :TO], AF.Relu,
                    bias=b1_t[:, l * NMT + mt: l * NMT + mt + 1])
                a1.append(a)

            res2 = []
            for mf in range(NFT):
                ps = mmtile()
                for kt2 in range(NMT):
                    wsrc = w2a if kt2 < 8 else w2b
                    koff = kt2 % 8
                    nc.tensor.matmul(
                        ps[:, :TO],
                        wsrc[:, koff * 512 + mf * 128: koff * 512 + (mf + 1) * 128],
                        a1[kt2][:],
                        start=(kt2 == 0), stop=(kt2 == NMT - 1))
                r = p.act.tile([128, TO], f32, tag=f"res2{mf}")
                nc.vector.tensor_scalar_add(
                    r[:], ps[:, :TO], b2_t[:, l * NFT + mf: l * NFT + mf + 1])
                nc.vector.tensor_add(r[:], r[:], ln1f[mf][:])
                res2.append(r)

            ht, htb = _layernorm(nc, p, ones_f, ones_r, res2,
                                 g2_t, be2_t, l, "ln2", mmtile)

        # ================= token-sharded vocab projection =================
        # out[t, v] = sum_e h[e, t] * wout[e, v] for the core's own 256 tokens.
        # h blocks are the stationary operand (reused across 4 chunk matmuls
        # per LDWEIGHTS); wout streams from HBM chunk by chunk.
        def wchunk(c):
            wt = p.wout.tile([128, NFT * NV], bf16, tag="wout")
            nc.sync.dma_start(wt[:], io["wout"][c])
            return wt

        r = 0
        for cg in range(NVC // 4):
            cs = [cg * 4 + i for i in range(4)]
            wtiles = [wchunk(c) for c in cs]
            for tt in range(NTT):
                # 4 chunks share one stationary h block per kf pass, so the
                # PE issues 4 streaming matmuls per LDWEIGHTS; the two PSUM
                # bank quads (uaug / mm+misc) ping-pong across rounds.
                pss = []
                for i in range(4):
                    if r % 2 == 0:
                        pss.append(p.ua.tile([128, NV], f32, tag="uaug",
                                             name=f"pj{r}_{i}"))
                    elif i < 3:
                        pss.append(p.mm.tile([128, 512], f32, tag="mm",
                                             name=f"pj{r}_{i}"))
                    else:
                        pss.append(p.misc.tile([128, NV], f32, tag="psmisc",
                                               name=f"pj{r}_{i}"))
                for kf in range(NFT):
                    for i in range(4):
                        nc.tensor.matmul(
                            pss[i][:, :NV],
                            htb[kf][:, tt * 128:(tt + 1) * 128],
                            wtiles[i][:, kf * NV:(kf + 1) * NV],
                            start=(kf == 0), stop=(kf == NFT - 1),
                            skip_group_check=True)
                for i, c in enumerate(cs):
                    ot = p.out.tile([128, NV], bf16, tag="outsb")
                    if i % 2 == 0:
                        nc.vector.tensor_copy(ot[:], pss[i][:, :NV])
                    else:
                        nc.scalar.mul(ot[:], pss[i][:, :NV], 1.0)
                    nc.sync.dma_start(io["out"][tt, c], ot[:])
                r += 1


def _layernorm(nc, p, ones_f, ones_r, res, g_t, b_t, l, name, mmtile):
    """Feature-major layernorm over NFT [128, TO] fp32 tiles -> (f32, bf16)."""
    sums = p.misc.tile([33, TO], f32, tag="psmisc")
    for kf in range(NFT):
        nc.tensor.matmul(sums[0:1, :], ones_f[:, :], res[kf][:],
                         start=(kf == 0), stop=(kf == NFT - 1))
    for kf in range(NFT):
        sq = p.act.tile([128, TO], f32, tag="sq", bufs=2)
        nc.scalar.activation(sq[:], res[kf][:], AF.Square)
        nc.tensor.matmul(sums[32:33, :], ones_f[:, :], sq[:],
                         start=(kf == 0), stop=(kf == NFT - 1))
    sv = p.stat.tile([1, 6 * TO], f32, tag="stat")
    mu = sv[:, 0:TO]
    musq = sv[:, TO:2 * TO]
    var = sv[:, 2 * TO:3 * TO]
    std = sv[:, 3 * TO:4 * TO]
    rstd = sv[:, 4 * TO:5 * TO]
    murstd = sv[:, 5 * TO:6 * TO]
    nc.scalar.mul(mu, sums[0:1, :], 1.0 / E)
    nc.vector.tensor_mul(musq, mu, mu)
    nc.vector.tensor_scalar(var, sums[32:33, :], 1.0 / E, EPS,
                            ALU.mult, ALU.add)
    nc.vector.tensor_sub(var, var, musq)
    # rstd = exp(-0.5*ln(var)): keeps ACT on the natural_log_exp table set
    # (a scalar.sqrt here would force a table swap against attention's Exp)
    nc.scalar.activation(std, var, AF.Ln)
    nc.scalar.activation(rstd, std, AF.Exp, scale=-0.5)
    nc.vector.tensor_mul(murstd, mu, rstd)
    rb = mmtile()
    nc.tensor.matmul(rb[:, :TO], ones_r[:, :], rstd, start=True, stop=True)
    mb = mmtile()
    nc.tensor.matmul(mb[:, :TO], ones_r[:, :], murstd, start=True, stop=True)
    outf, outb = [], []
    for kf in range(NFT):
        t = p.act.tile([128, TO], f32, tag=f"{name}f{kf}", bufs=2)
        nc.vector.tensor_mul(t[:], res[kf][:], rb[:, :TO])
        nc.vector.tensor_sub(t[:], t[:], mb[:, :TO])
        nc.vector.tensor_scalar(
            t[:], t[:],
            g_t[:, l * NFT + kf: l * NFT + kf + 1],
            b_t[:, l * NFT + kf: l * NFT + kf + 1],
            ALU.mult, ALU.add)
        b = p.act.tile([128, TO], bf16, tag=f"{name}b{kf}", bufs=2)
        nc.vector.tensor_copy(b[:], t[:])
        outf.append(t)
        outb.append(b)
    return outf, outb


def _prep_inputs(x, tok_emb, pos_emb, Wq, bq, Wk, bk, Wv, bv, Wo, bo,
                 W1, b1, W2, b2, ln1_g, ln1_b, ln2_g, ln2_b, Wout, bout):
    """Host-side sharding: returns in_maps for the 8 cores."""
    x = np.asarray(x)
    h0 = np.asarray(tok_emb)[x] + np.asarray(pos_emb)[None, :, :]   # [B,S,E] f32
    h0t = np.ascontiguousarray(np.transpose(h0, (0, 2, 1)))          # [B,E,S]

    scale = 1.0 / np.sqrt(D)
    wq_h = (np.transpose(np.asarray(Wq), (0, 2, 1, 3)).reshape(L, E, H * D)
            * scale).astype(BF16)
    wk_h = np.transpose(np.asarray(Wk), (0, 2, 1, 3)).reshape(L, E, H * D).astype(BF16)
    wv_h = np.transpose(np.asarray(Wv), (0, 2, 1, 3)).reshape(L, E, H * D).astype(BF16)
    wo_h = np.asarray(Wo).astype(BF16)
    w1_h = np.asarray(W1).astype(BF16)
    w2_h = np.asarray(W2).astype(BF16)
    bq_h = (np.asarray(bq).reshape(L, H * D) * scale).astype(np.float32)
    bk_h = np.asarray(bk).reshape(L, H * D).astype(np.float32)
    bv_c = np.asarray(bv).reshape(L, H * D).astype(np.float32)
    bo_eff = (np.asarray(bo) + np.einsum("lc,lce->le", bv_c, np.asarray(Wo))
              ).astype(np.float32)
    # pack wout into per-chunk on-chip tile layout: [c][p][k*NV+n] =
    # Wout[k*128+p, c*NV+n] -> contiguous 4KB DMA lines
    wout_np = np.ascontiguousarray(
        np.asarray(Wout).astype(BF16)
        .reshape(NFT, 128, NVC, NV)
        .transpose(2, 1, 0, 3)
        .reshape(NVC, 128, NFT * NV))
    common = dict(
        wq=wq_h, wk=wk_h, wv=wv_h, wo=wo_h, w1=w1_h, w2=w2_h,
        bq=bq_h, bk=bk_h, bo=bo_eff,
        b1=np.asarray(b1).astype(np.float32),
        b2=np.asarray(b2).astype(np.float32),
        g1=np.asarray(ln1_g).astype(np.float32),
        be1=np.asarray(ln1_b).astype(np.float32),
        g2=np.asarray(ln2_g).astype(np.float32),
        be2=np.asarray(ln2_b).astype(np.float32),
        wout=wout_np,
    )

    key_pos = np.arange(S)[:, None]
    in_maps = []
    for c in range(NC):
        b, j = c // G, c % G
        qpos = j * TO + np.arange(TO)[None, :]
        mask = (key_pos <= qpos).astype(BF16)            # [S, TO]
        maskown = np.ascontiguousarray(mask[j * TO:(j + 1) * TO, :])
        # gathered-path visibility per 128-key block: fully visible (0.0)
        # only strictly below this core's own rows; own rows come from the
        # local pass, everything else exp(-30)-masked
        maskb = np.full((128, NKT), -30.0, np.float32)
        maskb[:, :2 * j] = 0.0
        in_maps.append(dict(
            common,
            h0t_full=h0t[b].astype(BF16),
            h0t_own=np.ascontiguousarray(
                h0t[b][:, j * TO:(j + 1) * TO]).astype(np.float32),
            maskF=mask,
            maskB=maskb,
            maskOwn=maskown,
        ))
    return in_maps


def _finish_output(res, bout):
    bout = np.asarray(bout, dtype=np.float32)
    logits = np.empty((B, S, V), dtype=np.float32)
    for c in range(NC):
        b, j = c // G, c % G
        o = np.asarray(res.results[c]["out"], dtype=np.float32)
        o = o.transpose(0, 2, 1, 3).reshape(TO, V)   # [tt,c,p,n] -> [t, v]
        logits[b, j * TO:(j + 1) * TO, :] = o + bout[None, :]
    return logits


def kernel(**inputs):
    if "nc" not in _cache:
        _cache["nc"] = build_nc()
    nc = _cache["nc"]
    inputs = {k: np.asarray(v) for k, v in inputs.items()}
    in_maps = _prep_inputs(**inputs)
    res = run_bass_kernel_spmd(nc, in_maps, list(range(NC)))
    return _finish_output(res, inputs["bout"])



# revision 42
# speedup vs baseline: 1.1397x; 1.1397x over previous
"""GPT forward pass on 8 Trainium2 NeuronCores.

Sharding: cores 0-3 handle batch 0, cores 4-7 batch 1; within each 4-core
group the 1024 tokens are sequence-sharded 256/core. Activations are kept
feature-major (transposed) on chip. Per layer each core computes Q/K/V for
its own tokens; K and V are AllGathered in ONE fused bf16 collective within
the 4-core group (layer 0 computes full K/V from h0 directly, no
collective). Attention exponentials are batched 4-heads-at-a-time over
2-bank PSUM tiles. The final vocab projection is VOCAB-sharded: after an
8-core AllGather of the final hidden state, each core projects all 2048
tokens against its own 4000 vocab columns with Wout fully prefetched in
SBUF; the output bias is added on the host.
"""

import os
import sys

for _p in ("/opt/trn_rl_repo", "/root/.axon_site/_ro/trn_rl_repo"):
    if os.path.isdir(_p) and _p not in sys.path:
        sys.path.insert(0, _p)

import ml_dtypes
import numpy as np

import concourse.bass as bass
import concourse.mybir as mybir
import concourse.tile as tile
from concourse import bacc
from concourse.bass_utils import run_bass_kernel_spmd

BF16 = ml_dtypes.bfloat16
f32 = mybir.dt.float32
bf16 = mybir.dt.bfloat16
AF = mybir.ActivationFunctionType
ALU = mybir.AluOpType

V, S, E, H, D, L = 32000, 1024, 512, 8, 64, 4
FF = 4 * E
B = 2
NC = 8
G = 4            # cores per batch group
TO = S // G      # tokens owned per core (256)
EPS = 1e-5
NKT = S // 128   # key tiles (8)
NFT = E // 128   # feature tiles (4)
NTT = TO // 128  # own-token tiles (2)
NMT = FF // 128  # FFN hidden tiles (16)
VS = V // NC     # vocab columns per core (4000)
VSP = 4096       # padded vocab shard
NTB = (B * S) // 128   # token blocks in vocab phase (16)

_cache = {}
COLLFREE = False


def _pin_act_tables():
    """Force every activation function this kernel uses into the single
    `natural_log_exp_and_others` table set so the compiler never emits a
    mid-kernel ACT_TABLE_LOAD swap (each swap costs ~2.7us on ScalarE).
    The set genuinely contains ln/exp/square/relu/copy/identity."""
    import concourse.hw_specs as hw_specs

    if getattr(hw_specs, "_act_tables_pinned", False):
        return
    orig = hw_specs.get_activation_tables

    import functools

    @functools.cache
    def patched(module_arch):
        tabs = {k: set(v) for k, v in orig(module_arch).items()}
        combo = "natural_log_exp_and_others"
        if combo not in tabs:
            return tabs
        keep = tabs[combo]
        for name, fns in tabs.items():
            if name != combo:
                fns -= keep
        return tabs

    hw_specs.get_activation_tables = patched
    bacc.get_activation_tables = patched
    hw_specs._act_tables_pinned = True


def build_nc(trace=False, rep=1):
    if not os.environ.get("K_NO_ACTPIN"):
        _pin_act_tables()
    nc = bacc.Bacc("TRN2", target_bir_lowering=False, debug=False,
                   num_devices=1 if COLLFREE else NC)

    def din(name, shape, dt):
        return nc.dram_tensor(name, shape, dt, kind="ExternalInput").ap()

    io = dict(
        h0t_full=din("h0t_full", [E, S], bf16),
        h0t_own=din("h0t_own", [E, TO], f32),
        maskB=din("maskB", [128, NKT], f32),
        maskOwnR=din("maskOwnR", [NTT * 128, 4 * TO], bf16),
        wq=din("wq", [L, E, H * D], bf16),
        wk=din("wk", [L, E, H * D], bf16),
        wv=din("wv", [L, E, H * D], bf16),
        wo=din("wo", [L, H * D, E], bf16),
        w1=din("w1", [L, E, FF], bf16),
        w2=din("w2", [L, FF, E], bf16),
        bq=din("bq", [L, E], f32),
        bk=din("bk", [L, E], f32),
        bo=din("bo", [L, E], f32),
        b1=din("b1", [L, FF], f32),
        b2=din("b2", [L, E], f32),
        g1=din("g1", [L, E], f32),
        be1=din("be1", [L, E], f32),
        g2=din("g2", [L, E], f32),
        be2=din("be2", [L, E], f32),
        wout=din("wout", [NFT, 128, VSP], bf16),
        out=nc.dram_tensor("out", [NTB, 128, VSP], bf16,
                           kind="ExternalOutput").ap(),
    )

    with tile.TileContext(nc) as tc:
        _body(nc, tc, io, rep=rep)
    nc.compile()
    return nc


class P:
    """pool handles"""


def _body(nc, tc, io, rep=1):
    from contextlib import ExitStack

    ctx = ExitStack()
    with ctx:
        p = P()
        p.const = ctx.enter_context(tc.tile_pool(name="const", bufs=1))
        p.w512 = ctx.enter_context(tc.tile_pool(name="w512", bufs=4))
        p.wff = ctx.enter_context(tc.tile_pool(name="wff", bufs=3))
        p.kv = ctx.enter_context(tc.tile_pool(name="kv", bufs=1))
        p.hg = ctx.enter_context(tc.tile_pool(name="hg", bufs=4))
        p.act = ctx.enter_context(tc.tile_pool(name="act", bufs=1))
        p.es = ctx.enter_context(tc.tile_pool(name="esp", bufs=3))
        p.stat = ctx.enter_context(tc.tile_pool(name="stat", bufs=2))
        p.out = ctx.enter_context(tc.tile_pool(name="pout", bufs=2))
        p.ps = ctx.enter_context(tc.tile_pool(name="ps", bufs=4, space="PSUM"))
        p.dram = ctx.enter_context(tc.tile_pool(name="dram", bufs=2, space="DRAM"))

        _psn = [0]

        def pstile(name=None):
            if name is None:
                _psn[0] += 1
                name = f"ps{_psn[0]}"
            return p.ps.tile([128, 1024], f32, tag="ps", name=name)

        # ---- constants ----
        ones_f = p.const.tile([128, 1], bf16, tag="ones_f")
        nc.vector.memset(ones_f[:], 1.0)
        ones_r = p.const.tile([1, 128], f32, tag="ones_r")
        nc.vector.memset(ones_r[:], 1.0)

        def ldvec(ap, name, n=NFT):
            t = p.const.tile([128, L * n], f32, tag=name)
            nc.sync.dma_start(t[:], ap.rearrange("l (k p) -> p (l k)", p=128)[:])
            return t

        bq_t = ldvec(io["bq"], "bq")
        bk_t = ldvec(io["bk"], "bk")
        bo_t = ldvec(io["bo"], "bo")
        b2_t = ldvec(io["b2"], "b2")
        g1_t = ldvec(io["g1"], "g1")
        be1_t = ldvec(io["be1"], "be1")
        g2_t = ldvec(io["g2"], "g2")
        be2_t = ldvec(io["be2"], "be2")
        b1_t = ldvec(io["b1"], "b1", n=NMT)

        maskb_t = p.const.tile([128, NKT], f32, tag="maskb")
        nc.sync.dma_start(maskb_t[:], io["maskB"][:, :])
        moR = []
        for lt in range(NTT):
            m = p.const.tile([128, 4 * TO], bf16, tag=f"moR{lt}")
            nc.sync.dma_start(m[:], io["maskOwnR"][lt * 128:(lt + 1) * 128, :])
            moR.append(m)

        # Wout shard: fully resident in SBUF for the whole kernel
        wout_sb = []
        for kf in range(NFT):
            t = p.const.tile([128, VSP], bf16, tag=f"wout{kf}")
            nc.sync.dma_start(t[:], io["wout"][kf])
            wout_sb.append(t)

        # full h0 (bf16, feature-major) for layer-0 K/V
        h0f = []
        for kf in range(NFT):
            t = p.hg.tile([128, S], bf16, tag="hg", name=f"h0f{kf}")
            nc.sync.dma_start(t[:], io["h0t_full"][kf * 128:(kf + 1) * 128, :])
            h0f.append(t)

        # initial hidden state (own tokens, f32 quad [128, NFT*TO]);
        # shares tags with the per-layer LN2 outputs
        ht = p.act.tile([128, NFT * TO], f32, tag="lnf", bufs=3)
        nc.sync.dma_start(
            ht[:].rearrange("p (k c) -> p k c", k=NFT),
            io["h0t_own"].rearrange("(k p) c -> p k c", p=128)[:])
        htb = p.act.tile([128, NFT * TO], bf16, tag="lnb", bufs=3)
        nc.vector.tensor_copy(htb[:], ht[:])

        # persistent K^T / V tiles (gathered); ones columns set once
        kt_all = []
        for mf in range(NFT):
            kt = p.kv.tile([128, S], bf16, tag=f"kt{mf}")
            kt_all.append(kt)
        vt_all = []
        for tt in range(NKT):
            vt = p.kv.tile([128, H * 65], bf16, tag=f"vt{tt}")
            nc.vector.memset(
                vt.rearrange("p (h e) -> p h e", h=H)[:, :, 64:65], 1.0)
            vt_all.append(vt)
        vtloc = []
        for lt in range(NTT):
            vl = p.kv.tile([128, H * 65], bf16, tag=f"vtloc{lt}")
            nc.vector.memset(
                vl.rearrange("p (h e) -> p h e", h=H)[:, :, 64:65], 1.0)
            vtloc.append(vl)

        def load_w512(ap, l):
            t = p.w512.tile([128, NFT * 512], bf16, tag="w512")
            nc.sync.dma_start(t[:].rearrange("p (k n) -> p k n", k=NFT),
                              ap[l].rearrange("(k p) n -> p k n", p=128)[:])
            return t

        env = dict(locals())
        for _rep in range(rep):
            _compute(nc, tc, io, p, env)


def _compute(nc, tc, io, p, env):
    pstile = env["pstile"]
    ones_f = env["ones_f"]; ones_r = env["ones_r"]
    bq_t = env["bq_t"]; bk_t = env["bk_t"]; bo_t = env["bo_t"]; b2_t = env["b2_t"]
    g1_t = env["g1_t"]; be1_t = env["be1_t"]; g2_t = env["g2_t"]; be2_t = env["be2_t"]
    b1_t = env["b1_t"]; maskb_t = env["maskb_t"]; moR = env["moR"]
    wout_sb = env["wout_sb"]; h0f = env["h0f"]
    kt_all = env["kt_all"]; vt_all = env["vt_all"]; vtloc = env["vtloc"]
    load_w512 = env["load_w512"]
    ht = env["ht"]; htb = env["htb"]

    NLAYERS = int(os.environ.get("K_NL", L))
    SKIP_VOCAB = bool(os.environ.get("K_NOVOCAB"))
    STAGE = int(os.environ.get("K_STAGE", 99))

    def bail(t):
        nc.sync.dma_start(io["out"][0][:, 0:t.shape[-1]], t[:])

    # ================= transformer layers =================
    for l in range(NLAYERS):
        wq_t = load_w512(io["wq"], l)
        wk_t = load_w512(io["wk"], l)
        wv_t = load_w512(io["wv"], l)

        hb = htb  # bf16 activations of this layer's input

        # ---- K/V for own tokens (feeds local pass; l>0 also the AG) ----
        psK = pstile(f"psK{l}")
        for mf in range(NFT):
            for kf in range(NFT):
                nc.tensor.matmul(
                    psK[:, mf * 256:(mf + 1) * 256],
                    wk_t[:, kf * 512 + mf * 128: kf * 512 + (mf + 1) * 128],
                    hb[:, kf * 256:(kf + 1) * 256],
                    start=(kf == 0), stop=(kf == NFT - 1))
        klocal = p.act.tile([128, NFT * 256], bf16, tag="klocal", bufs=2)
        for mf in range(NFT):
            nc.vector.tensor_scalar_add(
                klocal[:, mf * 256:(mf + 1) * 256],
                psK[:, mf * 256:(mf + 1) * 256],
                bk_t[:, l * NFT + mf: l * NFT + mf + 1])

        psV = pstile(f"psV{l}")
        for tt in range(NTT):
            for kf in range(NFT):
                nc.tensor.matmul(
                    psV[:, tt * 512:(tt + 1) * 512],
                    hb[:, kf * 256 + tt * 128: kf * 256 + tt * 128 + 128],
                    wv_t[:, kf * 512:(kf + 1) * 512],
                    start=(kf == 0), stop=(kf == NFT - 1))
        vlocal = p.act.tile([128, NTT * 512], bf16, tag="vlocal", bufs=2)
        for tt in range(NTT):
            nc.vector.tensor_copy(
                vlocal[:, tt * 512:(tt + 1) * 512],
                psV[:, tt * 512:(tt + 1) * 512])
            nc.vector.tensor_copy(
                vtloc[tt].rearrange("p (h e) -> p h e", h=H)[:, :, 0:64],
                psV[:, tt * 512:(tt + 1) * 512]
                .rearrange("p (h e) -> p h e", h=H)[:, :, :])

        if l > 0 and not COLLFREE:
            contrib = p.dram.tile([2 * E * TO], bf16, tag="contrib")
            nc.sync.dma_start(
                contrib[0:E * TO].rearrange("(m p c) -> p m c", p=128, m=NFT),
                klocal[:].rearrange("p (m c) -> p m c", m=NFT))
            nc.sync.dma_start(
                contrib[E * TO:2 * E * TO].rearrange(
                    "(t p e) -> p t e", p=128, t=NTT),
                vlocal[:].rearrange("p (t e) -> p t e", t=NTT))
            gath = p.dram.tile([G, 2 * E * TO], bf16, tag="gath")
            nc.gpsimd.collective_compute(
                "AllGather", ALU.bypass,
                replica_groups=[[0, 1, 2, 3], [4, 5, 6, 7]],
                ins=[contrib[:]], outs=[gath[:]])
        elif l > 0:
            contrib = p.dram.tile([2 * E * TO], bf16, tag="contrib")
            nc.sync.dma_start(
                contrib[0:E * TO].rearrange("(m p c) -> p m c", p=128, m=NFT),
                klocal[:].rearrange("p (m c) -> p m c", m=NFT))
            nc.sync.dma_start(
                contrib[E * TO:2 * E * TO].rearrange(
                    "(t p e) -> p t e", p=128, t=NTT),
                vlocal[:].rearrange("p (t e) -> p t e", t=NTT))
            gath = p.dram.tile([G, 2 * E * TO], bf16, tag="gath")
            for rr in range(G):
                nc.sync.dma_start(gath[rr], contrib[:])

        # ---- Q^T (1/sqrt(D) folded into wq/bq on host) ----
        psQ = pstile(f"psQ{l}")
        for mf in range(NFT):
            for kf in range(NFT):
                nc.tensor.matmul(
                    psQ[:, mf * 256:(mf + 1) * 256],
                    wq_t[:, kf * 512 + mf * 128: kf * 512 + (mf + 1) * 128],
                    hb[:, kf * 256:(kf + 1) * 256],
                    start=(kf == 0), stop=(kf == NFT - 1))
        qt = p.act.tile([128, NFT * 256], bf16, tag="qt", bufs=2)
        for mf in range(NFT):
            nc.vector.tensor_scalar_add(
                qt[:, mf * 256:(mf + 1) * 256],
                psQ[:, mf * 256:(mf + 1) * 256],
                bq_t[:, l * NFT + mf: l * NFT + mf + 1])

        if STAGE == 1:
            bail(qt)
            return

        # prefetch remaining layer weights (overlaps attention)
        wo_t = load_w512(io["wo"], l)
        w1a = p.wff.tile([128, 4096], bf16, tag="wff")
        nc.sync.dma_start(
            w1a[:].rearrange("p (k n) -> p k n", k=4),
            io["w1"][l][:, 0:1024].rearrange("(k p) n -> p k n", p=128)[:])
        w1b = p.wff.tile([128, 4096], bf16, tag="wff")
        nc.sync.dma_start(
            w1b[:].rearrange("p (k n) -> p k n", k=4),
            io["w1"][l][:, 1024:2048].rearrange("(k p) n -> p k n", p=128)[:])

        # ---- gathered K/V: l==0 computes from full h0; l>0 loads AG ----
        if l == 0:
            for mf in range(NFT):
                psD = pstile(f"psD{mf}")
                for c2 in range(S // 512):
                    for kf in range(NFT):
                        nc.tensor.matmul(
                            psD[:, c2 * 512:(c2 + 1) * 512],
                            wk_t[:, kf * 512 + mf * 128: kf * 512 + (mf + 1) * 128],
                            h0f[kf][:, c2 * 512:(c2 + 1) * 512],
                            start=(kf == 0), stop=(kf == NFT - 1))
                nc.vector.tensor_scalar_add(
                    kt_all[mf][:], psD[:],
                    bk_t[:, l * NFT + mf: l * NFT + mf + 1])
            for tp in range(NKT // 2):
                psE = pstile(f"psE{tp}")
                for half in range(2):
                    tt8 = tp * 2 + half
                    for kf in range(NFT):
                        nc.tensor.matmul(
                            psE[:, half * 512:(half + 1) * 512],
                            h0f[kf][:, tt8 * 128:(tt8 + 1) * 128],
                            wv_t[:, kf * 512:(kf + 1) * 512],
                            start=(kf == 0), stop=(kf == NFT - 1))
                for half in range(2):
                    tt8 = tp * 2 + half
                    nc.vector.tensor_copy(
                        vt_all[tt8].rearrange("p (h e) -> p h e", h=H)[:, :, 0:64],
                        psE[:, half * 512:(half + 1) * 512]
                        .rearrange("p (h e) -> p h e", h=H)[:, :, :])
        else:
            kg = gath[:, 0:E * TO].rearrange(
                "r (m p c) -> m p r c", m=NFT, p=128)
            for mf in range(NFT):
                nc.sync.dma_start(
                    kt_all[mf][:].rearrange("p (r c) -> p r c", r=G),
                    kg[mf])
            for tt8 in range(NKT):
                r, tt = tt8 // 2, tt8 % 2
                vsrc = gath[r][E * TO:2 * E * TO].rearrange(
                    "(t p e) -> t p e", t=NTT, p=128)[tt].rearrange(
                    "p (h e) -> p h e", h=H)
                nc.sync.dma_start(
                    vt_all[tt8].rearrange("p (h e) -> p h e", h=H)[:, :, 0:64],
                    vsrc[:])

        if STAGE == 2:
            bail(klocal)
            return

        ATT = int(os.environ.get("K_ATT", 0))
        if ATT:
            cpA = p.act.tile([128, 1024], bf16, tag="qt", name="cpA", bufs=2)
            sc = pstile("sctest")
            if ATT == 2:
                # row-paired MMs to DIFFERENT banks
                cols = [0, 512, 0, 512]
                tiles2 = [sc, sc, pstile("sctest2"), pstile("sctest2b")]
                for hh in range(4):
                    nc.tensor.matmul(
                        tiles2[hh][:, cols[hh]:cols[hh] + 256],
                        klocal[(hh % 2) * 64:(hh % 2) * 64 + 64,
                               (hh // 2) * 256:(hh // 2) * 256 + 128],
                        qt[(hh % 2) * 64:(hh % 2) * 64 + 64,
                           (hh // 2) * 256:(hh // 2) * 256 + 256],
                        start=True, stop=True)
                nc.vector.tensor_copy(cpA[:, 0:256], sc[:, 0:256])
                nc.vector.tensor_copy(cpA[:, 256:512], tiles2[2][:, 0:256])
            else:
                for hh in range(4):
                    nc.tensor.matmul(
                        sc[:, hh * 256:(hh + 1) * 256],
                        klocal[(hh % 2) * 64:(hh % 2) * 64 + 64,
                               (hh // 2) * 256:(hh // 2) * 256 + 128],
                        qt[(hh % 2) * 64:(hh % 2) * 64 + 64,
                           (hh // 2) * 256:(hh // 2) * 256 + 256],
                        start=True, stop=True)
                if ATT == 1:
                    nc.vector.tensor_copy(cpA[:], sc[:])
                elif ATT == 3:
                    nc.scalar.activation(cpA[:], sc[:], AF.Exp)
                elif ATT == 4:
                    nc.scalar.activation(cpA[:, 0:512], sc[:, 0:512], AF.Exp)
                    nc.scalar.activation(cpA[:, 512:1024], sc[:, 512:1024],
                                         AF.Exp)
            bail(cpA)
            return

        # ---- attention ----
        # upair quads: heads 0-3 in upA, 4-7 in upB; numerator rows 0-63,
        # denominator (ones-column of V) at row 64. First local-AV write per
        # bank uses start=True to clear stale has_written bits.
        upA = pstile(f"upA{l}")
        upB = pstile(f"upB{l}")
        up = [upA, upB]

        def scol(hh):
            # score-column layout: row-paired heads (hh even at partitions
            # 0-63, hh odd at 64-127) run CONCURRENTLY on the PE, so they
            # must drain into different PSUM banks
            return (hh % 2) * 512 + (hh // 2) * 256

        def attend(keysrc_fn, vsrc, nloc, mask_mul, bias_kt):
            """one 128-key block x 8 heads: scores -> exp -> AV"""
            for grp in range(2):        # head groups: 0-3 / 4-7
                sc = pstile()
                for hh in range(4):
                    h = grp * 4 + hh
                    lhsT = keysrc_fn(h)
                    nc.tensor.matmul(
                        sc[:, scol(hh):scol(hh) + 256],
                        lhsT,
                        qt[(h % 2) * 64:(h % 2) * 64 + 64,
                           (h // 2) * 256:(h // 2) * 256 + 256],
                        start=True, stop=True)
                es = p.es.tile([128, 1024], bf16, tag="es",
                               name=f"es{l}_{nloc}_{grp}")
                if bias_kt is None:
                    nc.scalar.activation(es[:], sc[:], AF.Exp)
                else:
                    nc.scalar.activation(
                        es[:], sc[:], AF.Exp,
                        bias=maskb_t[:, bias_kt:bias_kt + 1])
                if mask_mul is not None:
                    # mask is head-independent: same [128, 4*TO] tile for
                    # both head groups
                    nc.vector.tensor_mul(es[:], es[:], mask_mul[:])
                for hh in range(4):
                    h = grp * 4 + hh
                    nc.tensor.matmul(
                        up[grp][0:65, hh * 256:(hh + 1) * 256],
                        vsrc[:, h * 65:(h + 1) * 65],
                        es[:, scol(hh):scol(hh) + 256],
                        start=(nloc == 0 and hh % 2 == 0),
                        stop=(nloc == NTT + NKT - 1 and hh % 2 == 1),
                        skip_group_check=True)

        nloc = 0
        for lt in range(NTT):
            attend(lambda h, lt=lt: klocal[
                       (h % 2) * 64:(h % 2) * 64 + 64,
                       (h // 2) * 256 + lt * 128:(h // 2) * 256 + lt * 128 + 128],
                   vtloc[lt], nloc, moR[lt], None)
            nloc += 1
        for kti in range(NKT):
            attend(lambda h, kti=kti: kt_all[h // 2][
                       (h % 2) * 64:(h % 2) * 64 + 64,
                       kti * 128:(kti + 1) * 128],
                   vt_all[kti], nloc, None, kti)
            nloc += 1

        if STAGE == 3:
            cp3 = p.act.tile([128, 1024], bf16, tag="qt", name="cp3", bufs=2)
            nc.vector.tensor_copy(cp3[0:64, :], upA[0:64, :])
            nc.vector.tensor_copy(cp3[64:128, :], upB[0:64, :])
            bail(cp3)
            return

        # ---- normalize heads -> conc^T [E, TO] bf16 ----
        rec = p.stat.tile([1, 2048], f32, tag="rec", bufs=1)
        if os.environ.get("K_NO_RECIP_APPROX"):
            nc.vector.reciprocal(rec[:, 0:1024], upA[64:65, :])
            nc.vector.reciprocal(rec[:, 1024:2048], upB[64:65, :])
        else:
            # reciprocal_approx_fast misreads PSUM operands on HW: stage
            # the denominator rows through SBUF first
            den = p.stat.tile([1, 2048], f32, tag="den", bufs=1)
            nc.vector.tensor_copy(den[:, 0:1024], upA[64:65, :])
            nc.vector.tensor_copy(den[:, 1024:2048], upB[64:65, :])
            nc.vector.reciprocal_approx_fast(rec[:], den[:])
        rbq = [pstile(f"rbq{l}a"), pstile(f"rbq{l}b")]
        for grp in range(2):
            for hh in range(4):
                nc.tensor.matmul(
                    rbq[grp][0:64, hh * 256:(hh + 1) * 256],
                    ones_r[0:1, 0:64],
                    rec[:, grp * 1024 + hh * 256: grp * 1024 + (hh + 1) * 256],
                    start=True, stop=True)
        rbs = p.stat.tile([64, 2048], bf16, tag="rbs", bufs=1)
        nc.vector.tensor_copy(rbs[:, 0:1024], rbq[0][0:64, :])
        nc.vector.tensor_copy(rbs[:, 1024:2048], rbq[1][0:64, :])
        conc = p.act.tile([128, NFT * 256], bf16, tag="conc", bufs=2)
        for h in range(H):
            grp, hh = h // 4, h % 4
            nc.vector.tensor_mul(
                conc[(h % 2) * 64:(h % 2) * 64 + 64,
                     (h // 2) * 256:(h // 2) * 256 + 256],
                up[grp][0:64, hh * 256:(hh + 1) * 256],
                rbs[:, grp * 1024 + hh * 256: grp * 1024 + (hh + 1) * 256])

        w2a = p.wff.tile([128, 4096], bf16, tag="wff")
        nc.sync.dma_start(
            w2a[:].rearrange("p (k n) -> p k n", k=8),
            io["w2"][l][0:1024, :].rearrange("(k p) n -> p k n", p=128)[:])
        w2b = p.wff.tile([128, 4096], bf16, tag="wff")
        nc.sync.dma_start(
            w2b[:].rearrange("p (k n) -> p k n", k=8),
            io["w2"][l][1024:2048, :].rearrange("(k p) n -> p k n", p=128)[:])

        # ---- mha^T + residual + LN1 ----
        psW = pstile(f"psW{l}")
        for mf in range(NFT):
            for kf in range(NFT):
                nc.tensor.matmul(
                    psW[:, mf * 256:(mf + 1) * 256],
                    wo_t[:, kf * 512 + mf * 128: kf * 512 + (mf + 1) * 128],
                    conc[:, kf * 256:(kf + 1) * 256],
                    start=(kf == 0), stop=(kf == NFT - 1))
        res1 = p.act.tile([128, NFT * 256], f32, tag="res", bufs=2)
        for mf in range(NFT):
            nc.vector.tensor_scalar_add(
                res1[:, mf * 256:(mf + 1) * 256],
                psW[:, mf * 256:(mf + 1) * 256],
                bo_t[:, l * NFT + mf: l * NFT + mf + 1])
        nc.vector.tensor_add(res1[:], res1[:], ht[:])

        if STAGE == 4:
            bail(conc)
            return

        ln1f, ln1b = _layernorm(nc, p, ones_f, ones_r, res1,
                                g1_t, be1_t, l, "ln1", pstile)

        if STAGE == 5:
            bail(ln1b)
            return

        # ---- FFN ----
        a1t = []
        for ag in range(2):
            a1 = p.act.tile([128, 8 * 256], bf16, tag="a1", bufs=2)
            a1t.append(a1)
            for half in range(2):
                psA = pstile()
                for m4 in range(4):
                    mt = ag * 8 + half * 4 + m4
                    wsrc = w1a if mt < 8 else w1b
                    moff = mt % 8
                    for kf in range(NFT):
                        nc.tensor.matmul(
                            psA[:, m4 * 256:(m4 + 1) * 256],
                            wsrc[:, kf * 1024 + moff * 128: kf * 1024 + (moff + 1) * 128],
                            ln1b[:, kf * 256:(kf + 1) * 256],
                            start=(kf == 0), stop=(kf == NFT - 1))
                for m4 in range(4):
                    mt = ag * 8 + half * 4 + m4
                    dst = a1[:, (half * 4 + m4) * 256:(half * 4 + m4 + 1) * 256]
                    src = psA[:, m4 * 256:(m4 + 1) * 256]
                    bia = b1_t[:, l * NMT + mt: l * NMT + mt + 1]
                    if m4 % 2 == 0:
                        nc.vector.tensor_scalar(
                            dst, src, bia, 0.0, ALU.add, ALU.max)
                    else:
                        nc.scalar.activation(dst, src, AF.Relu, bias=bia)

        psR = pstile(f"psR{l}")
        for mf in range(NFT):
            for kt2 in range(NMT):
                wsrc = w2a if kt2 < 8 else w2b
                koff = kt2 % 8
                nc.tensor.matmul(
                    psR[:, mf * 256:(mf + 1) * 256],
                    wsrc[:, koff * 512 + mf * 128: koff * 512 + (mf + 1) * 128],
                    a1t[kt2 // 8][:, (kt2 % 8) * 256:(kt2 % 8 + 1) * 256],
                    start=(kt2 == 0), stop=(kt2 == NMT - 1))
        res2 = p.act.tile([128, NFT * 256], f32, tag="res", bufs=2)
        for mf in range(NFT):
            nc.vector.tensor_scalar_add(
                res2[:, mf * 256:(mf + 1) * 256],
                psR[:, mf * 256:(mf + 1) * 256],
                b2_t[:, l * NFT + mf: l * NFT + mf + 1])
        nc.vector.tensor_add(res2[:], res2[:], ln1f[:])

        ht, htb = _layernorm(nc, p, ones_f, ones_r, res2,
                             g2_t, be2_t, l, "ln2", pstile)

    if SKIP_VOCAB:
        nc.sync.dma_start(io["out"][0][:, 0:NFT * TO], htb[:])
        return

    # ================= vocab-sharded projection =================
    # AllGather the final hidden state (bf16, feature-major) across all 8
    # cores, then project all 2048 tokens against this core's 4096-padded
    # vocab shard with Wout already resident in SBUF.
    contribH = p.dram.tile([E * TO], bf16, tag="contribH")
    nc.sync.dma_start(
        contribH[:].rearrange("(m p c) -> p m c", p=128, m=NFT),
        htb[:].rearrange("p (m c) -> p m c", m=NFT))
    gathH = p.dram.tile([NC, E * TO], bf16, tag="gathH",
                        addr_space="Local" if COLLFREE else "Shared")
    if COLLFREE:
        for rr in range(NC):
            nc.sync.dma_start(gathH[rr], contribH[:])
    else:
        nc.gpsimd.collective_compute(
            "AllGather", ALU.bypass,
            replica_groups=[[0, 1, 2, 3, 4, 5, 6, 7]],
            ins=[contribH[:]], outs=[gathH[:]])
    htg = []
    hgv = gathH.rearrange("r (m p c) -> m p r c", m=NFT, p=128)
    for kf in range(NFT):
        t = p.hg.tile([128, NC * TO], bf16, tag="hg", name=f"htg{kf}")
        nc.sync.dma_start(t[:].rearrange("p (r c) -> p r c", r=NC), hgv[kf])
        htg.append(t)

    for tb in range(NTB):
        duos = [pstile() for _ in range(4)]
        for kf in range(NFT):
            for vc in range(VSP // 512):
                nc.tensor.matmul(
                    duos[vc // 2][:, (vc % 2) * 512:(vc % 2 + 1) * 512],
                    htg[kf][:, tb * 128:(tb + 1) * 128],
                    wout_sb[kf][:, vc * 512:(vc + 1) * 512],
                    start=(kf == 0), stop=(kf == NFT - 1))
        for half in range(2):
            ob = p.out.tile([128, VSP // 2], bf16, tag="ob")
            for v2 in range(4):
                vc = half * 4 + v2
                dst = ob[:, v2 * 512:(v2 + 1) * 512]
                src = duos[vc // 2][:, (vc % 2) * 512:(vc % 2 + 1) * 512]
                if vc % 2 == 0:
                    nc.vector.tensor_copy(dst, src)
                else:
                    nc.scalar.copy(dst, src)
            nc.sync.dma_start(
                io["out"][tb][:, half * (VSP // 2):(half + 1) * (VSP // 2)],
                ob[:])


def _layernorm(nc, p, ones_f, ones_r, res, g_t, b_t, l, name, pstile):
    """Feature-major layernorm over a [128, NFT*TO] f32 quad -> (f32, bf16).

    Statistics are computed from a bf16 copy so the partition-sum matmuls
    stream bf16 (fp32-moving matmuls are 4x slower on the PE). Both LN1 and
    LN2 outputs share the lnf/lnb tags (bufs=3) to bound SBUF."""
    resb = p.act.tile([128, NFT * 256], bf16, tag="resb", bufs=2)
    nc.vector.tensor_copy(resb[:], res[:])
    sq = p.act.tile([128, NFT * 256], bf16, tag="sq", bufs=2)
    nc.vector.tensor_mul(sq[:], resb[:], resb[:])
    stats = pstile(f"stats_{name}{l}")
    for kf in range(NFT):
        nc.tensor.matmul(stats[0:1, 0:256], ones_f[:, :],
                         resb[:, kf * 256:(kf + 1) * 256],
                         start=(kf == 0), stop=(kf == NFT - 1))
    for kf in range(NFT):
        nc.tensor.matmul(stats[32:33, 0:256], ones_f[:, :],
                         sq[:, kf * 256:(kf + 1) * 256],
                         start=(kf == 0), stop=(kf == NFT - 1))
    sv = p.stat.tile([1, 4 * TO], f32, tag="stat", bufs=1)
    mu = sv[:, 0:TO]
    musq = sv[:, TO:2 * TO]
    var = sv[:, 2 * TO:3 * TO]
    std = sv[:, TO:2 * TO]          # reuses musq slot (musq dead)
    rstd = sv[:, 3 * TO:4 * TO]
    murstd = sv[:, 2 * TO:3 * TO]   # reuses var slot (var dead)
    nc.scalar.mul(mu, stats[0:1, 0:256], 1.0 / E)
    nc.vector.tensor_mul(musq, mu, mu)
    nc.vector.tensor_scalar(var, stats[32:33, 0:256], 1.0 / E, EPS,
                            ALU.mult, ALU.add)
    nc.vector.tensor_sub(var, var, musq)
    # rstd = exp(-0.5*ln(var)); ln+exp live in the single pinned table set
    nc.scalar.activation(std, var, AF.Ln)
    nc.scalar.activation(rstd, std, AF.Exp, scale=-0.5)
    nc.vector.tensor_mul(murstd, mu, rstd)
    # rb/mb broadcasts into bank 1 of the stats psum tile (cols 512..1023)
    rb = stats[:, 512:768]
    mb = stats[:, 768:1024]
    nc.tensor.matmul(rb, ones_r[:, :], rstd, start=True, stop=True)
    nc.tensor.matmul(mb, ones_r[:, :], murstd, start=True, stop=True)
    outf = p.act.tile([128, NFT * 256], f32, tag="lnf", bufs=3,
                      name=f"{name}f{l}")
    outb = p.act.tile([128, NFT * 256], bf16, tag="lnb", bufs=3,
                      name=f"{name}b{l}")
    for kf in range(NFT):
        t = outf[:, kf * 256:(kf + 1) * 256]
        nc.vector.tensor_mul(t, res[:, kf * 256:(kf + 1) * 256], rb)
        nc.vector.tensor_sub(t, t, mb)
        nc.vector.tensor_scalar(
            t, t,
            g_t[:, l * NFT + kf: l * NFT + kf + 1],
            b_t[:, l * NFT + kf: l * NFT + kf + 1],
            ALU.mult, ALU.add)
    nc.vector.tensor_copy(outb[:], outf[:])
    return outf, outb


def _prep_inputs(x, tok_emb, pos_emb, Wq, bq, Wk, bk, Wv, bv, Wo, bo,
                 W1, b1, W2, b2, ln1_g, ln1_b, ln2_g, ln2_b, Wout, bout):
    """Host-side sharding: returns in_maps for the 8 cores."""
    x = np.asarray(x)
    h0 = np.asarray(tok_emb)[x] + np.asarray(pos_emb)[None, :, :]   # [B,S,E] f32
    h0t = np.ascontiguousarray(np.transpose(h0, (0, 2, 1)))          # [B,E,S]

    scale = 1.0 / np.sqrt(D)
    wq_h = (np.transpose(np.asarray(Wq), (0, 2, 1, 3)).reshape(L, E, H * D)
            * scale).astype(BF16)
    wk_h = np.transpose(np.asarray(Wk), (0, 2, 1, 3)).reshape(L, E, H * D).astype(BF16)
    wv_h = np.transpose(np.asarray(Wv), (0, 2, 1, 3)).reshape(L, E, H * D).astype(BF16)
    wo_h = np.asarray(Wo).astype(BF16)
    w1_h = np.asarray(W1).astype(BF16)
    w2_h = np.asarray(W2).astype(BF16)
    bq_h = (np.asarray(bq).reshape(L, H * D) * scale).astype(np.float32)
    bk_h = np.asarray(bk).reshape(L, H * D).astype(np.float32)
    bv_c = np.asarray(bv).reshape(L, H * D).astype(np.float32)
    bo_eff = (np.asarray(bo) + np.einsum("lc,lce->le", bv_c, np.asarray(Wo))
              ).astype(np.float32)
    wout_np = np.zeros((NFT, 128, NC, VSP), dtype=BF16)
    wfull = np.asarray(Wout).astype(BF16).reshape(NFT, 128, V)
    for c in range(NC):
        wout_np[:, :, c, :VS] = wfull[:, :, c * VS:(c + 1) * VS]
    common = dict(
        wq=wq_h, wk=wk_h, wv=wv_h, wo=wo_h, w1=w1_h, w2=w2_h,
        bq=bq_h, bk=bk_h, bo=bo_eff,
        b1=np.asarray(b1).astype(np.float32),
        b2=np.asarray(b2).astype(np.float32),
        g1=np.asarray(ln1_g).astype(np.float32),
        be1=np.asarray(ln1_b).astype(np.float32),
        g2=np.asarray(ln2_g).astype(np.float32),
        be2=np.asarray(ln2_b).astype(np.float32),
    )

    in_maps = []
    for c in range(NC):
        b, j = c // G, c % G
        # own-block causal mask, replicated across 4 heads (both head
        # groups reuse the same tile): [NTT*128, 4*TO]
        qpos = j * TO + np.arange(TO)[None, :]
        moR = np.zeros((NTT * 128, 4 * TO), dtype=BF16)
        for lt in range(NTT):
            kpos = j * TO + lt * 128 + np.arange(128)[:, None]
            m = (kpos <= qpos).astype(BF16)          # [128, TO]
            moR[lt * 128:(lt + 1) * 128] = np.tile(m, (1, 4))
        # gathered-path visibility per 128-key block: fully visible (0.0)
        # only strictly below this core's own rows; own rows come from the
        # local pass, everything else exp(-30)-masked
        maskb = np.full((128, NKT), -30.0, np.float32)
        maskb[:, :2 * j] = 0.0
        in_maps.append(dict(
            common,
            h0t_full=h0t[b].astype(BF16),
            h0t_own=np.ascontiguousarray(
                h0t[b][:, j * TO:(j + 1) * TO]).astype(np.float32),
            maskB=maskb,
            maskOwnR=moR,
            wout=np.ascontiguousarray(wout_np[:, :, c, :]),
        ))
    return in_maps


def _finish_output(res, bout):
    bout = np.asarray(bout, dtype=np.float32)
    logits = np.empty((B, S, V), dtype=np.float32)
    for c in range(NC):
        o = np.asarray(res.results[c]["out"], dtype=np.float32)  # [NTB,128,VSP]
        for tb in range(NTB):
            r = tb // 2
            bb, j = r // G, r % G
            t0 = j * TO + (tb % 2) * 128
            logits[bb, t0:t0 + 128, c * VS:(c + 1) * VS] = o[tb][:, :VS]
    logits += bout[None, None, :]
    return logits


def kernel(**inputs):
    if "nc" not in _cache:
        _cache["nc"] = build_nc()
    nc = _cache["nc"]
    inputs = {k: np.asarray(v) for k, v in inputs.items()}
    in_maps = _prep_inputs(**inputs)
    res = run_bass_kernel_spmd(nc, in_maps, list(range(NC)))
    return _finish_output(res, inputs["bout"])


# revision 65
# speedup vs baseline: 1.2738x; 1.1176x over previous
"""GPT forward pass on 8 Trainium2 NeuronCores.

Sharding: cores 0-3 handle batch 0, cores 4-7 batch 1; within each 4-core
group the 1024 tokens are sequence-sharded 256/core. Activations are kept
feature-major (transposed) on chip. Per layer each core computes Q/K/V for
its own tokens; K and V are AllGathered in ONE fused bf16 collective within
the 4-core group (layer 0 computes full K/V from h0 directly, no
collective). Attention exponentials are batched 4-heads-at-a-time over
2-bank PSUM tiles. The final vocab projection is VOCAB-sharded: after an
8-core AllGather of the final hidden state, each core projects all 2048
tokens against its own 4000 vocab columns with Wout fully prefetched in
SBUF; the output bias is added on the host.
"""

import os
import sys

for _p in ("/opt/trn_rl_repo", "/root/.axon_site/_ro/trn_rl_repo"):
    if os.path.isdir(_p) and _p not in sys.path:
        sys.path.insert(0, _p)

import ml_dtypes
import numpy as np

import concourse.bass as bass
import concourse.mybir as mybir
import concourse.tile as tile
from concourse import bacc
from concourse.bass_utils import run_bass_kernel_spmd

BF16 = ml_dtypes.bfloat16
f32 = mybir.dt.float32
bf16 = mybir.dt.bfloat16
AF = mybir.ActivationFunctionType
ALU = mybir.AluOpType

V, S, E, H, D, L = 32000, 1024, 512, 8, 64, 4
FF = 4 * E
B = 2
NC = 8
G = 4            # cores per batch group
TO = S // G      # tokens owned per core (256)
EPS = 1e-5
NKT = S // 128   # key tiles (8)
NFT = E // 128   # feature tiles (4)
NTT = TO // 128  # own-token tiles (2)
NMT = FF // 128  # FFN hidden tiles (16)
VS = V // NC     # vocab columns per core (4000)
VSP = 4096       # padded vocab shard
NTB = (B * S) // 128   # token blocks in vocab phase (16)

_cache = {}
COLLFREE = False


def _pin_act_tables():
    """Force every activation function this kernel uses into the single
    `natural_log_exp_and_others` table set so the compiler never emits a
    mid-kernel ACT_TABLE_LOAD swap (each swap costs ~2.7us on ScalarE).
    The set genuinely contains ln/exp/square/relu/copy/identity."""
    import concourse.hw_specs as hw_specs

    if getattr(hw_specs, "_act_tables_pinned", False):
        return
    orig = hw_specs.get_activation_tables

    import functools

    @functools.cache
    def patched(module_arch):
        tabs = {k: set(v) for k, v in orig(module_arch).items()}
        combo = "natural_log_exp_and_others"
        if combo not in tabs:
            return tabs
        keep = tabs[combo]
        for name, fns in tabs.items():
            if name != combo:
                fns -= keep
        return tabs

    hw_specs.get_activation_tables = patched
    bacc.get_activation_tables = patched
    hw_specs._act_tables_pinned = True


def build_nc(trace=False, rep=1):
    if not os.environ.get("K_NO_ACTPIN"):
        _pin_act_tables()
    nc = bacc.Bacc("TRN2", target_bir_lowering=False, debug=False,
                   num_devices=1 if COLLFREE else NC)

    def din(name, shape, dt):
        return nc.dram_tensor(name, shape, dt, kind="ExternalInput").ap()

    io = dict(
        h0t_full=din("h0t_full", [E, S], bf16),
        h0t_own=din("h0t_own", [E, TO], f32),
        bvec=din("bvec", [128, 8 * L * NFT + L * NMT + NKT], f32),
        maskOwnR=din("maskOwnR", [NTT * 128, 4 * TO], bf16),
        wq=din("wq", [L, E, H * D], bf16),
        wk=din("wk", [L, E, H * D], bf16),
        wv=din("wv", [L, E, H * D], bf16),
        wo=din("wo", [L, H * D, E], bf16),
        w1=din("w1", [L, E, FF], bf16),
        w2=din("w2", [L, FF, E], bf16),
        wout=din("wout", [NFT, 128, VSP], bf16),
        out=nc.dram_tensor("out", [NTB + NTT, 128, VSP], bf16,
                           kind="ExternalOutput").ap(),
    )

    with tile.TileContext(nc) as tc:
        _body(nc, tc, io, rep=rep)
    nc.compile()
    return nc


class P:
    """pool handles"""


def _body(nc, tc, io, rep=1):
    from contextlib import ExitStack

    ctx = ExitStack()
    with ctx:
        p = P()
        p.const = ctx.enter_context(tc.tile_pool(name="const", bufs=1))
        p.w512 = ctx.enter_context(tc.tile_pool(name="w512", bufs=4))
        p.wff = ctx.enter_context(tc.tile_pool(name="wff", bufs=3))
        p.kv = ctx.enter_context(tc.tile_pool(name="kv", bufs=1))
        p.hg = ctx.enter_context(tc.tile_pool(name="hg", bufs=4))
        p.act = ctx.enter_context(tc.tile_pool(name="act", bufs=1))
        p.es = ctx.enter_context(tc.tile_pool(name="esp", bufs=3))
        p.stat = ctx.enter_context(tc.tile_pool(name="stat", bufs=2))
        p.out = ctx.enter_context(tc.tile_pool(name="pout", bufs=2))
        p.ps = ctx.enter_context(tc.tile_pool(name="ps", bufs=4, space="PSUM"))
        p.dram = ctx.enter_context(tc.tile_pool(name="dram", bufs=2, space="DRAM"))

        _psn = [0]

        def pstile(name=None):
            if name is None:
                _psn[0] += 1
                name = f"ps{_psn[0]}"
            return p.ps.tile([128, 1024], f32, tag="ps", name=name)

        # ---- constants ----
        ones_f = p.const.tile([128, 1], bf16, tag="ones_f")
        nc.vector.memset(ones_f[:], 1.0)
        ones_r = p.const.tile([1, 128], f32, tag="ones_r")
        nc.vector.memset(ones_r[:], 1.0)

        # all per-feature bias/scale vectors + block mask, packed host-side
        # into one contiguous [128, 200] f32 tensor -> a single fast DMA
        NB = 8 * L * NFT + L * NMT + NKT
        bvec_t = p.const.tile([128, NB], f32, tag="bvec")
        nc.sync.dma_start(bvec_t[:], io["bvec"][:, :])
        _off = [0]

        def bslice(n):
            t = bvec_t[:, _off[0]:_off[0] + n]
            _off[0] += n
            return t

        bq_t = bslice(L * NFT)
        bk_t = bslice(L * NFT)
        bo_t = bslice(L * NFT)
        b2_t = bslice(L * NFT)
        g1_t = bslice(L * NFT)
        be1_t = bslice(L * NFT)
        g2_t = bslice(L * NFT)
        be2_t = bslice(L * NFT)
        b1_t = bslice(L * NMT)
        maskb_t = bslice(NKT)
        moR = []
        for lt in range(NTT):
            m = p.const.tile([128, 4 * TO], bf16, tag=f"moR{lt}")
            nc.sync.dma_start(m[:], io["maskOwnR"][lt * 128:(lt + 1) * 128, :])
            moR.append(m)

        # Wout shard: fully resident in SBUF for the whole kernel; the DMAs
        # are issued from inside layer 1 so layer-0 weights load first
        wout_sb = []
        for kf in range(NFT):
            t = p.const.tile([128, VSP], bf16, tag=f"wout{kf}")
            wout_sb.append(t)

        # prime the ncfw collective path for the 4-rank groups during the
        # startup window so layer 1's first real AllGather starts warm
        if not COLLFREE:
            warm_sb = p.const.tile([128, 16], bf16, tag="warm_sb")
            nc.vector.memset(warm_sb[:], 0.0)
            warm_in = p.dram.tile([2048], bf16, tag="warm_in", bufs=1)
            nc.sync.dma_start(
                warm_in[:].rearrange("(p c) -> p c", p=128), warm_sb[:])
            warm_out = p.dram.tile([G, 2048], bf16, tag="warm_out", bufs=1)
            nc.gpsimd.collective_compute(
                "AllGather", ALU.bypass,
                replica_groups=[[0, 1, 2, 3], [4, 5, 6, 7]],
                ins=[warm_in[:]], outs=[warm_out[:]])

        # full h0 (bf16, feature-major) for layer-0 K/V
        h0f = []
        for kf in range(NFT):
            t = p.hg.tile([128, S], bf16, tag="hg", name=f"h0f{kf}")
            nc.sync.dma_start(t[:], io["h0t_full"][kf * 128:(kf + 1) * 128, :])
            h0f.append(t)

        # initial hidden state (own tokens, f32 quad [128, NFT*TO]);
        # shares tags with the per-layer LN2 outputs
        ht = p.act.tile([128, NFT * TO], f32, tag="lnf", bufs=3)
        nc.sync.dma_start(
            ht[:].rearrange("p (k c) -> p k c", k=NFT),
            io["h0t_own"].rearrange("(k p) c -> p k c", p=128)[:])
        htb = p.act.tile([128, NFT * TO], bf16, tag="lnb", bufs=3)
        nc.vector.tensor_copy(htb[:], ht[:])

        # persistent K^T / V tiles (gathered); ones columns set once
        kt_all = []
        for mf in range(NFT):
            kt = p.kv.tile([128, S], bf16, tag=f"kt{mf}")
            kt_all.append(kt)
        vt_all = []
        for tt in range(NKT):
            vt = p.kv.tile([128, H * 65], bf16, tag=f"vt{tt}")
            nc.vector.memset(
                vt.rearrange("p (h e) -> p h e", h=H)[:, :, 64:65], 1.0)
            vt_all.append(vt)
        vtloc = []
        for lt in range(NTT):
            vl = p.kv.tile([128, H * 65], bf16, tag=f"vtloc{lt}")
            nc.vector.memset(
                vl.rearrange("p (h e) -> p h e", h=H)[:, :, 64:65], 1.0)
            vtloc.append(vl)

        def load_w512(ap, l):
            t = p.w512.tile([128, NFT * 512], bf16, tag="w512")
            nc.sync.dma_start(t[:].rearrange("p (k n) -> p k n", k=NFT),
                              ap[l].rearrange("(k p) n -> p k n", p=128)[:])
            return t

        env = dict(locals())
        for _rep in range(rep):
            _compute(nc, tc, io, p, env)


def _compute(nc, tc, io, p, env):
    pstile = env["pstile"]
    ones_f = env["ones_f"]; ones_r = env["ones_r"]
    bq_t = env["bq_t"]; bk_t = env["bk_t"]; bo_t = env["bo_t"]; b2_t = env["b2_t"]
    g1_t = env["g1_t"]; be1_t = env["be1_t"]; g2_t = env["g2_t"]; be2_t = env["be2_t"]
    b1_t = env["b1_t"]; maskb_t = env["maskb_t"]; moR = env["moR"]
    wout_sb = env["wout_sb"]; h0f = env["h0f"]
    kt_all = env["kt_all"]; vt_all = env["vt_all"]; vtloc = env["vtloc"]
    load_w512 = env["load_w512"]
    ht = env["ht"]; htb = env["htb"]

    NLAYERS = int(os.environ.get("K_NL", L))
    SKIP_VOCAB = bool(os.environ.get("K_NOVOCAB"))
    STAGE = int(os.environ.get("K_STAGE", 99))

    def bail(t):
        nc.sync.dma_start(io["out"][0][:, 0:t.shape[-1]], t[:])

    # ================= transformer layers =================
    for l in range(NLAYERS):
        wq_t = load_w512(io["wq"], l)
        wk_t = load_w512(io["wk"], l)
        wv_t = load_w512(io["wv"], l)
        if l == 1:
            # prefetch the SBUF-resident Wout shard now that layer-0
            # weights are already in flight
            for kf in range(NFT):
                nc.sync.dma_start(wout_sb[kf][:], io["wout"][kf])

        hb = htb  # bf16 activations of this layer's input

        # ---- K/V for own tokens (feeds local pass; l>0 also the AG) ----
        # kf-outer so matmuls start as soon as hb chunk 0 exists; only the
        # bank-opening matmul (even mf, kf 0) uses start=True -- a second
        # start=True in the same bank would clear the first chunk's
        # has_written bits mid-accumulation
        psK = pstile(f"psK{l}")
        for kf in range(NFT):
            for mf in range(NFT):
                nc.tensor.matmul(
                    psK[:, mf * 256:(mf + 1) * 256],
                    wk_t[:, kf * 512 + mf * 128: kf * 512 + (mf + 1) * 128],
                    hb[:, kf * 256:(kf + 1) * 256],
                    start=(kf == 0 and mf % 2 == 0),
                    stop=(kf == NFT - 1 and mf % 2 == 1),
                    skip_group_check=True)
        klocal = p.act.tile([128, NFT * 256], bf16, tag="klocal", bufs=2)
        for mf in range(NFT):
            nc.vector.tensor_scalar_add(
                klocal[:, mf * 256:(mf + 1) * 256],
                psK[:, mf * 256:(mf + 1) * 256],
                bk_t[:, l * NFT + mf: l * NFT + mf + 1])

        psV = pstile(f"psV{l}")
        for kf in range(NFT):
            for tt in range(NTT):
                nc.tensor.matmul(
                    psV[:, tt * 512:(tt + 1) * 512],
                    hb[:, kf * 256 + tt * 128: kf * 256 + tt * 128 + 128],
                    wv_t[:, kf * 512:(kf + 1) * 512],
                    start=(kf == 0), stop=(kf == NFT - 1))
        vlocal = p.act.tile([128, NTT * 512], bf16, tag="vlocal", bufs=2)
        for tt in range(NTT):
            nc.vector.tensor_copy(
                vlocal[:, tt * 512:(tt + 1) * 512],
                psV[:, tt * 512:(tt + 1) * 512])
            nc.vector.tensor_copy(
                vtloc[tt].rearrange("p (h e) -> p h e", h=H)[:, :, 0:64],
                psV[:, tt * 512:(tt + 1) * 512]
                .rearrange("p (h e) -> p h e", h=H)[:, :, :])

        if l > 0 and not COLLFREE:
            contrib = p.dram.tile([2 * E * TO], bf16, tag="contrib")
            nc.sync.dma_start(
                contrib[0:E * TO].rearrange("(m p c) -> p m c", p=128, m=NFT),
                klocal[:].rearrange("p (m c) -> p m c", m=NFT))
            nc.sync.dma_start(
                contrib[E * TO:2 * E * TO].rearrange(
                    "(t p e) -> p t e", p=128, t=NTT),
                vlocal[:].rearrange("p (t e) -> p t e", t=NTT))
            gath = p.dram.tile([G, 2 * E * TO], bf16, tag="gath")
            nc.gpsimd.collective_compute(
                "AllGather", ALU.bypass,
                replica_groups=[[0, 1, 2, 3], [4, 5, 6, 7]],
                ins=[contrib[:]], outs=[gath[:]])
        elif l > 0:
            contrib = p.dram.tile([2 * E * TO], bf16, tag="contrib")
            nc.sync.dma_start(
                contrib[0:E * TO].rearrange("(m p c) -> p m c", p=128, m=NFT),
                klocal[:].rearrange("p (m c) -> p m c", m=NFT))
            nc.sync.dma_start(
                contrib[E * TO:2 * E * TO].rearrange(
                    "(t p e) -> p t e", p=128, t=NTT),
                vlocal[:].rearrange("p (t e) -> p t e", t=NTT))
            gath = p.dram.tile([G, 2 * E * TO], bf16, tag="gath")
            for rr in range(G):
                nc.sync.dma_start(gath[rr], contrib[:])

        # ---- Q^T (1/sqrt(D) folded into wq/bq on host) ----
        psQ = pstile(f"psQ{l}")
        for kf in range(NFT):
            for mf in range(NFT):
                nc.tensor.matmul(
                    psQ[:, mf * 256:(mf + 1) * 256],
                    wq_t[:, kf * 512 + mf * 128: kf * 512 + (mf + 1) * 128],
                    hb[:, kf * 256:(kf + 1) * 256],
                    start=(kf == 0 and mf % 2 == 0),
                    stop=(kf == NFT - 1 and mf % 2 == 1),
                    skip_group_check=True)
        qt = p.act.tile([128, NFT * 256], bf16, tag="qt", bufs=2)
        for mf in range(NFT):
            nc.vector.tensor_scalar_add(
                qt[:, mf * 256:(mf + 1) * 256],
                psQ[:, mf * 256:(mf + 1) * 256],
                bq_t[:, l * NFT + mf: l * NFT + mf + 1])

        if STAGE == 1:
            bail(qt)
            return

        # prefetch remaining layer weights (overlaps attention)
        wo_t = load_w512(io["wo"], l)
        w1a = p.wff.tile([128, 4096], bf16, tag="wff")
        nc.sync.dma_start(
            w1a[:].rearrange("p (k n) -> p k n", k=4),
            io["w1"][l][:, 0:1024].rearrange("(k p) n -> p k n", p=128)[:])
        w1b = p.wff.tile([128, 4096], bf16, tag="wff")
        nc.sync.dma_start(
            w1b[:].rearrange("p (k n) -> p k n", k=4),
            io["w1"][l][:, 1024:2048].rearrange("(k p) n -> p k n", p=128)[:])

        # ---- gathered K/V: l==0 computes from full h0; l>0 loads AG ----
        if l == 0:
            for mf in range(NFT):
                psD = pstile(f"psD{mf}")
                for c2 in range(S // 512):
                    for kf in range(NFT):
                        nc.tensor.matmul(
                            psD[:, c2 * 512:(c2 + 1) * 512],
                            wk_t[:, kf * 512 + mf * 128: kf * 512 + (mf + 1) * 128],
                            h0f[kf][:, c2 * 512:(c2 + 1) * 512],
                            start=(kf == 0), stop=(kf == NFT - 1))
                nc.vector.tensor_scalar_add(
                    kt_all[mf][:], psD[:],
                    bk_t[:, l * NFT + mf: l * NFT + mf + 1])
            for tp in range(NKT // 2):
                psE = pstile(f"psE{tp}")
                for half in range(2):
                    tt8 = tp * 2 + half
                    for kf in range(NFT):
                        nc.tensor.matmul(
                            psE[:, half * 512:(half + 1) * 512],
                            h0f[kf][:, tt8 * 128:(tt8 + 1) * 128],
                            wv_t[:, kf * 512:(kf + 1) * 512],
                            start=(kf == 0), stop=(kf == NFT - 1))
                for half in range(2):
                    tt8 = tp * 2 + half
                    nc.vector.tensor_copy(
                        vt_all[tt8].rearrange("p (h e) -> p h e", h=H)[:, :, 0:64],
                        psE[:, half * 512:(half + 1) * 512]
                        .rearrange("p (h e) -> p h e", h=H)[:, :, :])
        else:
            kg = gath[:, 0:E * TO].rearrange(
                "r (m p c) -> m p r c", m=NFT, p=128)
            for mf in range(NFT):
                nc.sync.dma_start(
                    kt_all[mf][:].rearrange("p (r c) -> p r c", r=G),
                    kg[mf])
            for tt8 in range(NKT):
                r, tt = tt8 // 2, tt8 % 2
                vsrc = gath[r][E * TO:2 * E * TO].rearrange(
                    "(t p e) -> t p e", t=NTT, p=128)[tt].rearrange(
                    "p (h e) -> p h e", h=H)
                nc.sync.dma_start(
                    vt_all[tt8].rearrange("p (h e) -> p h e", h=H)[:, :, 0:64],
                    vsrc[:])

        if STAGE == 2:
            bail(klocal)
            return

        # ---- attention ----
        # upair quads: heads 0-3 in upA, 4-7 in upB; numerator rows 0-63,
        # denominator (ones-column of V) at row 64. First local-AV write per
        # bank uses start=True to clear stale has_written bits.
        upA = pstile(f"upA{l}")
        upB = pstile(f"upB{l}")
        up = [upA, upB]

        def scol(hh):
            # score-column layout: row-paired heads (hh even at partitions
            # 0-63, hh odd at 64-127) run CONCURRENTLY on the PE, so they
            # must drain into different PSUM banks
            return (hh % 2) * 512 + (hh // 2) * 256

        def attend(keysrc_fn, vsrc, nloc, mask_mul, bias_kt):
            """one 128-key block x 8 heads: scores -> exp -> AV"""
            for grp in range(2):        # head groups: 0-3 / 4-7
                sc = pstile()
                for hh in range(4):
                    h = grp * 4 + hh
                    lhsT = keysrc_fn(h)
                    nc.tensor.matmul(
                        sc[:, scol(hh):scol(hh) + 256],
                        lhsT,
                        qt[(h % 2) * 64:(h % 2) * 64 + 64,
                           (h // 2) * 256:(h // 2) * 256 + 256],
                        start=True, stop=True)
                es = p.es.tile([128, 1024], bf16, tag="es",
                               name=f"es{l}_{nloc}_{grp}")
                if bias_kt is None:
                    nc.scalar.activation(es[:], sc[:], AF.Exp)
                else:
                    nc.scalar.activation(
                        es[:], sc[:], AF.Exp,
                        bias=maskb_t[:, bias_kt:bias_kt + 1])
                if mask_mul is not None:
                    # mask is head-independent: same [128, 4*TO] tile for
                    # both head groups
                    nc.vector.tensor_mul(es[:], es[:], mask_mul[:])
                for hh in range(4):
                    h = grp * 4 + hh
                    nc.tensor.matmul(
                        up[grp][0:65, hh * 256:(hh + 1) * 256],
                        vsrc[:, h * 65:(h + 1) * 65],
                        es[:, scol(hh):scol(hh) + 256],
                        start=(nloc == 0 and hh % 2 == 0),
                        stop=(nloc == NTT + NKT - 1 and hh % 2 == 1),
                        skip_group_check=True)

        nloc = 0
        for lt in range(NTT):
            attend(lambda h, lt=lt: klocal[
                       (h % 2) * 64:(h % 2) * 64 + 64,
                       (h // 2) * 256 + lt * 128:(h // 2) * 256 + lt * 128 + 128],
                   vtloc[lt], nloc, moR[lt], None)
            nloc += 1
        for kti in range(NKT):
            attend(lambda h, kti=kti: kt_all[h // 2][
                       (h % 2) * 64:(h % 2) * 64 + 64,
                       kti * 128:(kti + 1) * 128],
                   vt_all[kti], nloc, None, kti)
            nloc += 1

        if STAGE == 3:
            cp3 = p.act.tile([128, 1024], bf16, tag="qt", name="cp3", bufs=2)
            nc.vector.tensor_copy(cp3[0:64, :], upA[0:64, :])
            nc.vector.tensor_copy(cp3[64:128, :], upB[0:64, :])
            bail(cp3)
            return

        # ---- normalize heads -> conc^T [E, TO] bf16 ----
        rec = p.stat.tile([1, 2048], f32, tag="rec", bufs=1)
        if os.environ.get("K_NO_RECIP_APPROX"):
            nc.vector.reciprocal(rec[:, 0:1024], upA[64:65, :])
            nc.vector.reciprocal(rec[:, 1024:2048], upB[64:65, :])
        else:
            # reciprocal_approx_fast misreads PSUM operands on HW: stage
            # the denominator rows through SBUF first
            den = p.stat.tile([1, 2048], f32, tag="den", bufs=1)
            nc.vector.tensor_copy(den[:, 0:1024], upA[64:65, :])
            nc.vector.tensor_copy(den[:, 1024:2048], upB[64:65, :])
            nc.vector.reciprocal_approx_fast(rec[:], den[:])
        rbq = [pstile(f"rbq{l}a"), pstile(f"rbq{l}b")]
        for grp in range(2):
            for hh in range(4):
                nc.tensor.matmul(
                    rbq[grp][0:64, hh * 256:(hh + 1) * 256],
                    ones_r[0:1, 0:64],
                    rec[:, grp * 1024 + hh * 256: grp * 1024 + (hh + 1) * 256],
                    start=True, stop=True)
        rbs = p.stat.tile([64, 2048], bf16, tag="rbs", bufs=1)
        nc.vector.tensor_copy(rbs[:, 0:1024], rbq[0][0:64, :])
        nc.vector.tensor_copy(rbs[:, 1024:2048], rbq[1][0:64, :])
        conc = p.act.tile([128, NFT * 256], bf16, tag="conc", bufs=2)
        for h in range(H):
            grp, hh = h // 4, h % 4
            nc.vector.tensor_mul(
                conc[(h % 2) * 64:(h % 2) * 64 + 64,
                     (h // 2) * 256:(h // 2) * 256 + 256],
                up[grp][0:64, hh * 256:(hh + 1) * 256],
                rbs[:, grp * 1024 + hh * 256: grp * 1024 + (hh + 1) * 256])

        w2a = p.wff.tile([128, 4096], bf16, tag="wff")
        nc.sync.dma_start(
            w2a[:].rearrange("p (k n) -> p k n", k=8),
            io["w2"][l][0:1024, :].rearrange("(k p) n -> p k n", p=128)[:])
        w2b = p.wff.tile([128, 4096], bf16, tag="wff")
        nc.sync.dma_start(
            w2b[:].rearrange("p (k n) -> p k n", k=8),
            io["w2"][l][1024:2048, :].rearrange("(k p) n -> p k n", p=128)[:])

        # ---- mha^T + residual + LN1 ----
        psW = pstile(f"psW{l}")
        for mf in range(NFT):
            for kf in range(NFT):
                nc.tensor.matmul(
                    psW[:, mf * 256:(mf + 1) * 256],
                    wo_t[:, kf * 512 + mf * 128: kf * 512 + (mf + 1) * 128],
                    conc[:, kf * 256:(kf + 1) * 256],
                    start=(kf == 0), stop=(kf == NFT - 1))
        res1 = p.act.tile([128, NFT * 256], f32, tag="res", bufs=2)
        for mf in range(NFT):
            nc.vector.tensor_scalar_add(
                res1[:, mf * 256:(mf + 1) * 256],
                psW[:, mf * 256:(mf + 1) * 256],
                bo_t[:, l * NFT + mf: l * NFT + mf + 1])
        nc.vector.tensor_add(res1[:], res1[:], ht[:])

        if STAGE == 4:
            bail(conc)
            return

        ln1f, ln1b = _layernorm(nc, p, ones_f, ones_r, res1,
                                g1_t, be1_t, l, "ln1", pstile)

        if STAGE == 5:
            bail(ln1b)
            return

        # ---- FFN ----
        a1t = []
        for ag in range(2):
            a1 = p.act.tile([128, 8 * 256], bf16, tag="a1", bufs=2)
            a1t.append(a1)
            for half in range(2):
                psA = pstile()
                for kf in range(NFT):
                    for m4 in range(4):
                        mt = ag * 8 + half * 4 + m4
                        wsrc = w1a if mt < 8 else w1b
                        moff = mt % 8
                        nc.tensor.matmul(
                            psA[:, m4 * 256:(m4 + 1) * 256],
                            wsrc[:, kf * 1024 + moff * 128: kf * 1024 + (moff + 1) * 128],
                            ln1b[:, kf * 256:(kf + 1) * 256],
                            start=(kf == 0 and m4 % 2 == 0),
                            stop=(kf == NFT - 1 and m4 % 2 == 1),
                            skip_group_check=True)
                for m4 in range(4):
                    mt = ag * 8 + half * 4 + m4
                    dst = a1[:, (half * 4 + m4) * 256:(half * 4 + m4 + 1) * 256]
                    src = psA[:, m4 * 256:(m4 + 1) * 256]
                    bia = b1_t[:, l * NMT + mt: l * NMT + mt + 1]
                    if m4 % 2 == 0:
                        nc.vector.tensor_scalar(
                            dst, src, bia, 0.0, ALU.add, ALU.max)
                    else:
                        nc.scalar.activation(dst, src, AF.Relu, bias=bia)

        psR = pstile(f"psR{l}")
        for kt2 in range(NMT):
            wsrc = w2a if kt2 < 8 else w2b
            koff = kt2 % 8
            for mf in range(NFT):
                nc.tensor.matmul(
                    psR[:, mf * 256:(mf + 1) * 256],
                    wsrc[:, koff * 512 + mf * 128: koff * 512 + (mf + 1) * 128],
                    a1t[kt2 // 8][:, (kt2 % 8) * 256:(kt2 % 8 + 1) * 256],
                    start=(kt2 == 0 and mf % 2 == 0),
                    stop=(kt2 == NMT - 1 and mf % 2 == 1),
                    skip_group_check=True)
        res2 = p.act.tile([128, NFT * 256], f32, tag="res", bufs=2)
        for mf in range(NFT):
            nc.vector.tensor_scalar_add(
                res2[:, mf * 256:(mf + 1) * 256],
                psR[:, mf * 256:(mf + 1) * 256],
                b2_t[:, l * NFT + mf: l * NFT + mf + 1])
        nc.vector.tensor_add(res2[:], res2[:], ln1f[:])

        ht, htb = _layernorm(nc, p, ones_f, ones_r, res2,
                             g2_t, be2_t, l, "ln2", pstile)

    if SKIP_VOCAB:
        nc.sync.dma_start(io["out"][0][:, 0:NFT * TO], htb[:])
        return
    if NLAYERS < 2:
        for kf in range(NFT):
            nc.sync.dma_start(wout_sb[kf][:], io["wout"][kf])

    # ================= vocab-sharded projection =================
    # AllGather the final hidden state (bf16, feature-major) across all 8
    # cores, then project all 2048 tokens against this core's 4096-padded
    # vocab shard with Wout already resident in SBUF.
    contribH = p.dram.tile([E * TO], bf16, tag="contribH")
    nc.sync.dma_start(
        contribH[:].rearrange("(m p c) -> p m c", p=128, m=NFT),
        htb[:].rearrange("p (m c) -> p m c", m=NFT))
    gathH = p.dram.tile([NC, E * TO], bf16, tag="gathH",
                        addr_space="Local" if COLLFREE else "Shared")
    if COLLFREE:
        for rr in range(NC):
            nc.sync.dma_start(gathH[rr], contribH[:])
    else:
        nc.gpsimd.collective_compute(
            "AllGather", ALU.bypass,
            replica_groups=[[0, 1, 2, 3, 4, 5, 6, 7]],
            ins=[contribH[:]], outs=[gathH[:]])
    def project(tb_out, stat_fn):
        """project one 128-token block against the full vocab shard"""
        duos = [pstile() for _ in range(4)]
        for kf in range(NFT):
            for vc in range(VSP // 512):
                nc.tensor.matmul(
                    duos[vc // 2][:, (vc % 2) * 512:(vc % 2 + 1) * 512],
                    stat_fn(kf),
                    wout_sb[kf][:, vc * 512:(vc + 1) * 512],
                    start=(kf == 0), stop=(kf == NFT - 1))
        for half in range(2):
            ob = p.out.tile([128, VSP // 2], bf16, tag="ob")
            for v2 in range(4):
                vc = half * 4 + v2
                dst = ob[:, v2 * 512:(v2 + 1) * 512]
                src = duos[vc // 2][:, (vc % 2) * 512:(vc % 2 + 1) * 512]
                if vc % 2 == 0:
                    nc.vector.tensor_copy(dst, src)
                else:
                    nc.scalar.copy(dst, src)
            nc.sync.dma_start(
                io["out"][tb_out][:, half * (VSP // 2):(half + 1) * (VSP // 2)],
                ob[:])

    # own token blocks first, straight from local htb -- overlaps the
    # AllGather; the host uses slots NTB..NTB+1 for this core's rows
    for h2 in range(NTT):
        project(NTB + h2,
                lambda kf, h2=h2: htb[:, kf * 256 + h2 * 128:
                                      kf * 256 + h2 * 128 + 128])

    htg = []
    hgv = gathH.rearrange("r (m p c) -> m p r c", m=NFT, p=128)
    for kf in range(NFT):
        t = p.hg.tile([128, NC * TO], bf16, tag="hg", name=f"htg{kf}")
        nc.sync.dma_start(t[:].rearrange("p (r c) -> p r c", r=NC), hgv[kf])
        htg.append(t)

    for tb in range(NTB):
        project(tb, lambda kf, tb=tb: htg[kf][:, tb * 128:(tb + 1) * 128])


def _layernorm(nc, p, ones_f, ones_r, res, g_t, b_t, l, name, pstile):
    """Feature-major layernorm over a [128, NFT*TO] f32 quad -> (f32, bf16).

    Statistics are computed from a bf16 copy so the partition-sum matmuls
    stream bf16 (fp32-moving matmuls are 4x slower on the PE). Both LN1 and
    LN2 outputs share the lnf/lnb tags (bufs=3) to bound SBUF."""
    resb = p.act.tile([128, NFT * 256], bf16, tag="resb", bufs=2)
    nc.vector.tensor_copy(resb[:], res[:])
    sq = p.act.tile([128, NFT * 256], bf16, tag="sq", bufs=2)
    nc.vector.tensor_mul(sq[:], resb[:], resb[:])
    stats = pstile(f"stats_{name}{l}")
    for kf in range(NFT):
        nc.tensor.matmul(stats[0:1, 0:256], ones_f[:, :],
                         resb[:, kf * 256:(kf + 1) * 256],
                         start=(kf == 0), stop=(kf == NFT - 1))
    for kf in range(NFT):
        nc.tensor.matmul(stats[32:33, 0:256], ones_f[:, :],
                         sq[:, kf * 256:(kf + 1) * 256],
                         start=(kf == 0), stop=(kf == NFT - 1))
    sv = p.stat.tile([1, 4 * TO], f32, tag="stat", bufs=1)
    mu = sv[:, 0:TO]
    musq = sv[:, TO:2 * TO]
    var = sv[:, 2 * TO:3 * TO]
    std = sv[:, TO:2 * TO]          # reuses musq slot (musq dead)
    rstd = sv[:, 3 * TO:4 * TO]
    murstd = sv[:, 2 * TO:3 * TO]   # reuses var slot (var dead)
    nc.scalar.mul(mu, stats[0:1, 0:256], 1.0 / E)
    nc.vector.tensor_mul(musq, mu, mu)
    nc.vector.tensor_scalar(var, stats[32:33, 0:256], 1.0 / E, EPS,
                            ALU.mult, ALU.add)
    nc.vector.tensor_sub(var, var, musq)
    # rstd = exp(-0.5*ln(var)); ln+exp live in the single pinned table set
    nc.scalar.activation(std, var, AF.Ln)
    nc.scalar.activation(rstd, std, AF.Exp, scale=-0.5)
    nc.vector.tensor_mul(murstd, mu, rstd)
    # rb/mb broadcasts into bank 1 of the stats psum tile (cols 512..1023)
    rb = stats[:, 512:768]
    mb = stats[:, 768:1024]
    nc.tensor.matmul(rb, ones_r[:, :], rstd, start=True, stop=True)
    nc.tensor.matmul(mb, ones_r[:, :], murstd, start=True, stop=True)
    outf = p.act.tile([128, NFT * 256], f32, tag="lnf", bufs=3,
                      name=f"{name}f{l}")
    outb = p.act.tile([128, NFT * 256], bf16, tag="lnb", bufs=3,
                      name=f"{name}b{l}")
    for kf in range(NFT):
        t = outf[:, kf * 256:(kf + 1) * 256]
        nc.vector.tensor_mul(t, res[:, kf * 256:(kf + 1) * 256], rb)
        nc.vector.tensor_sub(t, t, mb)
        nc.vector.tensor_scalar(
            t, t,
            g_t[:, l * NFT + kf: l * NFT + kf + 1],
            b_t[:, l * NFT + kf: l * NFT + kf + 1],
            ALU.mult, ALU.add)
        # per-chunk bf16 cast so downstream matmuls start on chunk 0
        # while later chunks are still being normalized
        nc.vector.tensor_copy(outb[:, kf * 256:(kf + 1) * 256], t)
    return outf, outb


def _prep_inputs(x, tok_emb, pos_emb, Wq, bq, Wk, bk, Wv, bv, Wo, bo,
                 W1, b1, W2, b2, ln1_g, ln1_b, ln2_g, ln2_b, Wout, bout):
    """Host-side sharding: returns in_maps for the 8 cores."""
    x = np.asarray(x)
    h0 = np.asarray(tok_emb)[x] + np.asarray(pos_emb)[None, :, :]   # [B,S,E] f32
    h0t = np.ascontiguousarray(np.transpose(h0, (0, 2, 1)))          # [B,E,S]

    scale = 1.0 / np.sqrt(D)
    wq_h = (np.transpose(np.asarray(Wq), (0, 2, 1, 3)).reshape(L, E, H * D)
            * scale).astype(BF16)
    wk_h = np.transpose(np.asarray(Wk), (0, 2, 1, 3)).reshape(L, E, H * D).astype(BF16)
    wv_h = np.transpose(np.asarray(Wv), (0, 2, 1, 3)).reshape(L, E, H * D).astype(BF16)
    wo_h = np.asarray(Wo).astype(BF16)
    w1_h = np.asarray(W1).astype(BF16)
    w2_h = np.asarray(W2).astype(BF16)
    bq_h = (np.asarray(bq).reshape(L, H * D) * scale).astype(np.float32)
    bk_h = np.asarray(bk).reshape(L, H * D).astype(np.float32)
    bv_c = np.asarray(bv).reshape(L, H * D).astype(np.float32)
    bo_eff = (np.asarray(bo) + np.einsum("lc,lce->le", bv_c, np.asarray(Wo))
              ).astype(np.float32)
    wout_np = np.zeros((NFT, 128, NC, VSP), dtype=BF16)
    wfull = np.asarray(Wout).astype(BF16).reshape(NFT, 128, V)
    for c in range(NC):
        wout_np[:, :, c, :VS] = wfull[:, :, c * VS:(c + 1) * VS]

    # pack all [L, E]-style bias/scale vectors (+ the per-core key-block
    # mask appended later) into one [128, 200] f32 tensor: col l*n+k holds
    # arr[l, k*128+p] for partition p
    def pk(arr, n=NFT):
        return np.ascontiguousarray(
            np.asarray(arr, dtype=np.float32)
            .reshape(L, n, 128).transpose(2, 0, 1).reshape(128, L * n))

    bvec_c = np.concatenate([
        pk(bq_h), pk(bk_h), pk(bo_eff), pk(b2),
        pk(ln1_g), pk(ln1_b), pk(ln2_g), pk(ln2_b),
        pk(b1, n=NMT)], axis=1)

    common = dict(
        wq=wq_h, wk=wk_h, wv=wv_h, wo=wo_h, w1=w1_h, w2=w2_h,
    )

    in_maps = []
    for c in range(NC):
        b, j = c // G, c % G
        # own-block causal mask, replicated across 4 heads (both head
        # groups reuse the same tile): [NTT*128, 4*TO]
        qpos = j * TO + np.arange(TO)[None, :]
        moR = np.zeros((NTT * 128, 4 * TO), dtype=BF16)
        for lt in range(NTT):
            kpos = j * TO + lt * 128 + np.arange(128)[:, None]
            m = (kpos <= qpos).astype(BF16)          # [128, TO]
            moR[lt * 128:(lt + 1) * 128] = np.tile(m, (1, 4))
        # gathered-path visibility per 128-key block: fully visible (0.0)
        # only strictly below this core's own rows; own rows come from the
        # local pass, everything else exp(-30)-masked
        maskb = np.full((128, NKT), -30.0, np.float32)
        maskb[:, :2 * j] = 0.0
        in_maps.append(dict(
            common,
            h0t_full=h0t[b].astype(BF16),
            h0t_own=np.ascontiguousarray(
                h0t[b][:, j * TO:(j + 1) * TO]).astype(np.float32),
            bvec=np.ascontiguousarray(
                np.concatenate([bvec_c, maskb], axis=1)),
            maskOwnR=moR,
            wout=np.ascontiguousarray(wout_np[:, :, c, :]),
        ))
    return in_maps


def _finish_output(res, bout):
    bout = np.asarray(bout, dtype=np.float32)
    logits = np.empty((B, S, V), dtype=np.float32)
    for c in range(NC):
        o = np.asarray(res.results[c]["out"], dtype=np.float32)  # [NTB+2,...]
        for tb in range(NTB):
            r = tb // 2
            bb, j = r // G, r % G
            t0 = j * TO + (tb % 2) * 128
            # own token blocks come from the early (pre-AllGather) slots
            src = NTB + (tb % 2) if r == c else tb
            logits[bb, t0:t0 + 128, c * VS:(c + 1) * VS] = o[src][:, :VS]
    logits += bout[None, None, :]
    return logits


def kernel(**inputs):
    if "nc" not in _cache:
        _cache["nc"] = build_nc()
    nc = _cache["nc"]
    inputs = {k: np.asarray(v) for k, v in inputs.items()}
    in_maps = _prep_inputs(**inputs)
    res = run_bass_kernel_spmd(nc, in_maps, list(range(NC)))
    return _finish_output(res, inputs["bout"])


# revision 77
# speedup vs baseline: 1.3672x; 1.0734x over previous
"""GPT forward pass on 8 Trainium2 NeuronCores.

Sharding: cores 0-3 handle batch 0, cores 4-7 batch 1; within each 4-core
group the 1024 tokens are sequence-sharded 256/core. Activations are kept
feature-major (transposed) on chip. Per layer each core computes Q/K/V for
its own tokens; K and V are AllGathered in ONE fused bf16 collective within
the 4-core group (layer 0 computes full K/V from h0 directly, no
collective). Attention exponentials are batched 4-heads-at-a-time over
2-bank PSUM tiles. The final vocab projection is VOCAB-sharded: after an
8-core AllGather of the final hidden state, each core projects all 2048
tokens against its own 4000 vocab columns with Wout fully prefetched in
SBUF; the output bias is added on the host.
"""

import os
import sys

for _p in ("/opt/trn_rl_repo", "/root/.axon_site/_ro/trn_rl_repo"):
    if os.path.isdir(_p) and _p not in sys.path:
        sys.path.insert(0, _p)

import ml_dtypes
import numpy as np

import concourse.bass as bass
import concourse.mybir as mybir
import concourse.tile as tile
from concourse import bacc
from concourse.bass_utils import run_bass_kernel_spmd

BF16 = ml_dtypes.bfloat16
f32 = mybir.dt.float32
bf16 = mybir.dt.bfloat16
AF = mybir.ActivationFunctionType
ALU = mybir.AluOpType

V, S, E, H, D, L = 32000, 1024, 512, 8, 64, 4
FF = 4 * E
B = 2
NC = 8
G = 4            # cores per batch group
TO = S // G      # tokens owned per core (256)
EPS = 1e-5
NKT = S // 128   # key tiles (8)
NGT = NKT - 2    # gathered key tiles actually attended (6): tiles 6,7 are
                 # above every core's own rows -> always exp(-30)-masked
NFT = E // 128   # feature tiles (4)
NTT = TO // 128  # own-token tiles (2)
NMT = FF // 128  # FFN hidden tiles (16)
VS = V // NC     # vocab columns per core (4000)
VSP = 4096       # padded vocab shard
NTB = (B * S) // 128   # token blocks in vocab phase (16)

_cache = {}
COLLFREE = False


def _pin_act_tables():
    """Force every activation function this kernel uses into the single
    `natural_log_exp_and_others` table set so the compiler never emits a
    mid-kernel ACT_TABLE_LOAD swap (each swap costs ~2.7us on ScalarE).
    The set genuinely contains ln/exp/square/relu/copy/identity."""
    import concourse.hw_specs as hw_specs

    if getattr(hw_specs, "_act_tables_pinned", False):
        return
    orig = hw_specs.get_activation_tables

    import functools

    @functools.cache
    def patched(module_arch):
        tabs = {k: set(v) for k, v in orig(module_arch).items()}
        combo = "natural_log_exp_and_others"
        if combo not in tabs:
            return tabs
        keep = tabs[combo]
        for name, fns in tabs.items():
            if name != combo:
                fns -= keep
        return tabs

    hw_specs.get_activation_tables = patched
    bacc.get_activation_tables = patched
    hw_specs._act_tables_pinned = True


def build_nc(trace=False, rep=1):
    if not os.environ.get("K_NO_ACTPIN"):
        _pin_act_tables()
    nc = bacc.Bacc("TRN2", target_bir_lowering=False, debug=False,
                   num_devices=1 if COLLFREE else NC)

    def din(name, shape, dt):
        return nc.dram_tensor(name, shape, dt, kind="ExternalInput").ap()

    io = dict(
        h0t_full=din("h0t_full", [E, S], bf16),
        h0t_own=din("h0t_own", [E, TO], f32),
        bvec=din("bvec", [128, 8 * L * NFT + L * NMT + NKT], f32),
        maskOwnR=din("maskOwnR", [NTT * 128, 4 * TO], bf16),
        wq=din("wq", [L, E, H * D], bf16),
        wk=din("wk", [L, E, H * D], bf16),
        wv=din("wv", [L, E, H * D], bf16),
        wo=din("wo", [L, H * D, E], bf16),
        w1=din("w1", [L, E, FF], bf16),
        w2=din("w2", [L, FF, E], bf16),
        wout=din("wout", [NFT, 128, VSP], bf16),
        out=nc.dram_tensor("out", [NTB + NTT, 128, VSP], bf16,
                           kind="ExternalOutput").ap(),
    )

    with tile.TileContext(nc) as tc:
        _body(nc, tc, io, rep=rep)
    nc.compile()
    return nc


class P:
    """pool handles"""


def _body(nc, tc, io, rep=1):
    from contextlib import ExitStack

    ctx = ExitStack()
    with ctx:
        p = P()
        p.const = ctx.enter_context(tc.tile_pool(name="const", bufs=1))
        p.w512 = ctx.enter_context(tc.tile_pool(name="w512", bufs=4))
        p.wff = ctx.enter_context(tc.tile_pool(name="wff", bufs=3))
        p.kv = ctx.enter_context(tc.tile_pool(name="kv", bufs=1))
        p.hg = ctx.enter_context(tc.tile_pool(name="hg", bufs=4))
        p.act = ctx.enter_context(tc.tile_pool(name="act", bufs=1))
        p.es = ctx.enter_context(tc.tile_pool(name="esp", bufs=3))
        p.stat = ctx.enter_context(tc.tile_pool(name="stat", bufs=2))
        p.out = ctx.enter_context(tc.tile_pool(name="pout", bufs=2))
        p.ps = ctx.enter_context(tc.tile_pool(name="ps", bufs=4, space="PSUM"))
        p.dram = ctx.enter_context(tc.tile_pool(name="dram", bufs=2, space="DRAM"))

        _psn = [0]

        def pstile(name=None):
            if name is None:
                _psn[0] += 1
                name = f"ps{_psn[0]}"
            return p.ps.tile([128, 1024], f32, tag="ps", name=name)

        # ---- constants ----
        ones_f = p.const.tile([128, 1], bf16, tag="ones_f")
        nc.vector.memset(ones_f[:], 1.0)
        ones_r = p.const.tile([1, 128], f32, tag="ones_r")
        nc.vector.memset(ones_r[:], 1.0)

        # all per-feature bias/scale vectors + block mask, packed host-side
        # into one contiguous [128, 200] f32 tensor -> a single fast DMA
        NB = 8 * L * NFT + L * NMT + NKT
        bvec_t = p.const.tile([128, NB], f32, tag="bvec")
        nc.sync.dma_start(bvec_t[:], io["bvec"][:, :])
        _off = [0]

        def bslice(n):
            t = bvec_t[:, _off[0]:_off[0] + n]
            _off[0] += n
            return t

        bq_t = bslice(L * NFT)
        bk_t = bslice(L * NFT)
        bo_t = bslice(L * NFT)
        b2_t = bslice(L * NFT)
        g1_t = bslice(L * NFT)
        be1_t = bslice(L * NFT)
        g2_t = bslice(L * NFT)
        be2_t = bslice(L * NFT)
        b1_t = bslice(L * NMT)
        maskb_t = bslice(NKT)
        # moR/h0f DMAs are issued from inside layer 0, after the layer-0
        # weight loads, to get the PE started as early as possible
        moR = []
        for lt in range(NTT):
            m = p.const.tile([128, 4 * TO], bf16, tag=f"moR{lt}")
            moR.append(m)

        # Wout shard: fully resident in SBUF for the whole kernel; the DMAs
        # are issued from inside layer 1 so layer-0 weights load first
        wout_sb = []
        for kf in range(NFT):
            t = p.const.tile([128, VSP], bf16, tag=f"wout{kf}")
            wout_sb.append(t)

        # prime the ncfw collective path for the 4-rank groups during the
        # startup window so layer 1's first real AllGather starts warm
        if not COLLFREE:
            warm_sb = p.const.tile([128, 16], bf16, tag="warm_sb")
            nc.vector.memset(warm_sb[:], 0.0)
            warm_in = p.dram.tile([2048], bf16, tag="warm_in", bufs=1)
            nc.sync.dma_start(
                warm_in[:].rearrange("(p c) -> p c", p=128), warm_sb[:])
            warm_out = p.dram.tile([G, 2048], bf16, tag="warm_out", bufs=1)
            nc.gpsimd.collective_compute(
                "AllGather", ALU.bypass,
                replica_groups=[[0, 1, 2, 3], [4, 5, 6, 7]],
                ins=[warm_in[:]], outs=[warm_out[:]])

        # full h0 (bf16, feature-major) for layer-0 K/V
        h0f = []
        for kf in range(NFT):
            t = p.hg.tile([128, S], bf16, tag="hg", name=f"h0f{kf}")
            h0f.append(t)

        # initial hidden state (own tokens, f32 quad [128, NFT*TO]);
        # shares tags with the per-layer LN2 outputs
        ht = p.act.tile([128, NFT * TO], f32, tag="lnf", bufs=3)
        nc.sync.dma_start(
            ht[:].rearrange("p (k c) -> p k c", k=NFT),
            io["h0t_own"].rearrange("(k p) c -> p k c", p=128)[:])
        htb = p.act.tile([128, NFT * TO], bf16, tag="lnb", bufs=3)
        nc.vector.tensor_copy(htb[:], ht[:])

        # persistent K^T / V tiles (gathered, key tiles 0..NGT-1 only);
        # ones columns set once
        kt_all = []
        for mf in range(NFT):
            kt = p.kv.tile([128, NGT * 128], bf16, tag=f"kt{mf}")
            kt_all.append(kt)
        vt_all = []
        for tt in range(NGT):
            vt = p.kv.tile([128, H * 65], bf16, tag=f"vt{tt}")
            nc.vector.memset(
                vt.rearrange("p (h e) -> p h e", h=H)[:, :, 64:65], 1.0)
            vt_all.append(vt)
        vtloc = []
        for lt in range(NTT):
            vl = p.kv.tile([128, H * 65], bf16, tag=f"vtloc{lt}")
            nc.vector.memset(
                vl.rearrange("p (h e) -> p h e", h=H)[:, :, 64:65], 1.0)
            vtloc.append(vl)

        def load_w512(ap, l):
            t = p.w512.tile([128, NFT * 512], bf16, tag="w512")
            nc.sync.dma_start(t[:].rearrange("p (k n) -> p k n", k=NFT),
                              ap[l].rearrange("(k p) n -> p k n", p=128)[:])
            return t

        env = dict(locals())
        for _rep in range(rep):
            _compute(nc, tc, io, p, env)


def _compute(nc, tc, io, p, env):
    pstile = env["pstile"]
    ones_f = env["ones_f"]; ones_r = env["ones_r"]
    bq_t = env["bq_t"]; bk_t = env["bk_t"]; bo_t = env["bo_t"]; b2_t = env["b2_t"]
    g1_t = env["g1_t"]; be1_t = env["be1_t"]; g2_t = env["g2_t"]; be2_t = env["be2_t"]
    b1_t = env["b1_t"]; maskb_t = env["maskb_t"]; moR = env["moR"]
    wout_sb = env["wout_sb"]; h0f = env["h0f"]
    kt_all = env["kt_all"]; vt_all = env["vt_all"]; vtloc = env["vtloc"]
    load_w512 = env["load_w512"]
    ht = env["ht"]; htb = env["htb"]

    NLAYERS = int(os.environ.get("K_NL", L))
    SKIP_VOCAB = bool(os.environ.get("K_NOVOCAB"))
    STAGE = int(os.environ.get("K_STAGE", 99))
    AG_BF16 = bool(os.environ.get("K_AG_BF16"))

    def bail(t):
        nc.sync.dma_start(io["out"][0][:, 0:t.shape[-1]], t[:])

    # ================= transformer layers =================
    for l in range(NLAYERS):
        wq_t = load_w512(io["wq"], l)
        wk_t = load_w512(io["wk"], l)
        wv_t = load_w512(io["wv"], l)
        if l == 0:
            for kf in range(NFT):
                nc.sync.dma_start(h0f[kf][:],
                                  io["h0t_full"][kf * 128:(kf + 1) * 128, :])
            for lt in range(NTT):
                nc.sync.dma_start(
                    moR[lt][:], io["maskOwnR"][lt * 128:(lt + 1) * 128, :])
        if l == 1:
            # prefetch the SBUF-resident Wout shard now that layer-0
            # weights are already in flight
            for kf in range(NFT):
                nc.sync.dma_start(wout_sb[kf][:], io["wout"][kf])

        hb = htb  # bf16 activations of this layer's input

        # ---- K/V for own tokens (feeds local pass; l>0 also the AG) ----
        # kf-outer so matmuls start as soon as hb chunk 0 exists; only the
        # bank-opening matmul (even mf, kf 0) uses start=True -- a second
        # start=True in the same bank would clear the first chunk's
        # has_written bits mid-accumulation
        psK = pstile(f"psK{l}")
        for kf in range(NFT):
            for mf in range(NFT):
                nc.tensor.matmul(
                    psK[:, mf * 256:(mf + 1) * 256],
                    wk_t[:, kf * 512 + mf * 128: kf * 512 + (mf + 1) * 128],
                    hb[:, kf * 256:(kf + 1) * 256],
                    start=(kf == 0 and mf % 2 == 0),
                    stop=(kf == NFT - 1 and mf % 2 == 1),
                    skip_group_check=True)
        klocal = p.act.tile([128, NFT * 256], bf16, tag="klocal", bufs=2)
        for mf in range(NFT):
            nc.vector.tensor_scalar_add(
                klocal[:, mf * 256:(mf + 1) * 256],
                psK[:, mf * 256:(mf + 1) * 256],
                bk_t[:, l * NFT + mf: l * NFT + mf + 1])

        psV = pstile(f"psV{l}")
        for kf in range(NFT):
            for tt in range(NTT):
                nc.tensor.matmul(
                    psV[:, tt * 512:(tt + 1) * 512],
                    hb[:, kf * 256 + tt * 128: kf * 256 + tt * 128 + 128],
                    wv_t[:, kf * 512:(kf + 1) * 512],
                    start=(kf == 0), stop=(kf == NFT - 1))
        vlocal = p.act.tile([128, NTT * 512], bf16, tag="vlocal", bufs=2)
        for tt in range(NTT):
            nc.vector.tensor_copy(
                vlocal[:, tt * 512:(tt + 1) * 512],
                psV[:, tt * 512:(tt + 1) * 512])
            nc.vector.tensor_copy(
                vtloc[tt].rearrange("p (h e) -> p h e", h=H)[:, :, 0:64],
                psV[:, tt * 512:(tt + 1) * 512]
                .rearrange("p (h e) -> p h e", h=H)[:, :, :])

        if l > 0:
            # K/V travel the AllGather in fp8e4m3 (half the wire bytes of
            # bf16 -- the 4-rank mesh AG is bandwidth-limited); the SWDGE
            # DMAs cast on the way out and back in
            wdt = bf16 if AG_BF16 else mybir.dt.float8e4
            dma_cast = nc.sync.dma_start if AG_BF16 else nc.gpsimd.dma_start
            contrib = p.dram.tile([2 * E * TO], wdt, tag="contrib")
            dma_cast(
                out=contrib[0:E * TO].rearrange("(m p c) -> p m c",
                                                p=128, m=NFT),
                in_=klocal[:].rearrange("p (m c) -> p m c", m=NFT))
            dma_cast(
                out=contrib[E * TO:2 * E * TO].rearrange(
                    "(t p e) -> p t e", p=128, t=NTT),
                in_=vlocal[:].rearrange("p (t e) -> p t e", t=NTT))
            gath = p.dram.tile([G, 2 * E * TO], wdt, tag="gath")
            if COLLFREE:
                for rr in range(G):
                    nc.sync.dma_start(gath[rr], contrib[:])
            else:
                nc.gpsimd.collective_compute(
                    "AllGather", ALU.bypass,
                    replica_groups=[[0, 1, 2, 3], [4, 5, 6, 7]],
                    ins=[contrib[:]], outs=[gath[:]])

        # ---- Q^T (1/sqrt(D) folded into wq/bq on host) ----
        psQ = pstile(f"psQ{l}")
        for kf in range(NFT):
            for mf in range(NFT):
                nc.tensor.matmul(
                    psQ[:, mf * 256:(mf + 1) * 256],
                    wq_t[:, kf * 512 + mf * 128: kf * 512 + (mf + 1) * 128],
                    hb[:, kf * 256:(kf + 1) * 256],
                    start=(kf == 0 and mf % 2 == 0),
                    stop=(kf == NFT - 1 and mf % 2 == 1),
                    skip_group_check=True)
        qt = p.act.tile([128, NFT * 256], bf16, tag="qt", bufs=2)
        for mf in range(NFT):
            nc.vector.tensor_scalar_add(
                qt[:, mf * 256:(mf + 1) * 256],
                psQ[:, mf * 256:(mf + 1) * 256],
                bq_t[:, l * NFT + mf: l * NFT + mf + 1])

        if STAGE == 1:
            bail(qt)
            return

        # prefetch remaining layer weights (overlaps attention)
        wo_t = load_w512(io["wo"], l)
        w1a = p.wff.tile([128, 4096], bf16, tag="wff")
        nc.sync.dma_start(
            w1a[:].rearrange("p (k n) -> p k n", k=4),
            io["w1"][l][:, 0:1024].rearrange("(k p) n -> p k n", p=128)[:])
        w1b = p.wff.tile([128, 4096], bf16, tag="wff")
        nc.sync.dma_start(
            w1b[:].rearrange("p (k n) -> p k n", k=4),
            io["w1"][l][:, 1024:2048].rearrange("(k p) n -> p k n", p=128)[:])

        # ---- gathered K/V: l==0 computes from full h0; l>0 loads AG ----
        if l == 0:
            for mf in range(NFT):
                psD = pstile(f"psD{mf}")
                for c2 in range(2):
                    cw = 512 if c2 == 0 else NGT * 128 - 512
                    for kf in range(NFT):
                        nc.tensor.matmul(
                            psD[:, c2 * 512:c2 * 512 + cw],
                            wk_t[:, kf * 512 + mf * 128: kf * 512 + (mf + 1) * 128],
                            h0f[kf][:, c2 * 512:c2 * 512 + cw],
                            start=(kf == 0), stop=(kf == NFT - 1))
                nc.vector.tensor_scalar_add(
                    kt_all[mf][:], psD[:, 0:NGT * 128],
                    bk_t[:, l * NFT + mf: l * NFT + mf + 1])
            for tp in range(NGT // 2):
                psE = pstile(f"psE{tp}")
                for half in range(2):
                    tt8 = tp * 2 + half
                    for kf in range(NFT):
                        nc.tensor.matmul(
                            psE[:, half * 512:(half + 1) * 512],
                            h0f[kf][:, tt8 * 128:(tt8 + 1) * 128],
                            wv_t[:, kf * 512:(kf + 1) * 512],
                            start=(kf == 0), stop=(kf == NFT - 1))
                for half in range(2):
                    tt8 = tp * 2 + half
                    nc.vector.tensor_copy(
                        vt_all[tt8].rearrange("p (h e) -> p h e", h=H)[:, :, 0:64],
                        psE[:, half * 512:(half + 1) * 512]
                        .rearrange("p (h e) -> p h e", h=H)[:, :, :])
        else:
            # only ranks 0..NGT/2-1 feed the attended gathered tiles
            dma_cast = nc.sync.dma_start if AG_BF16 else nc.gpsimd.dma_start
            kg = gath[:, 0:E * TO].rearrange(
                "r (m p c) -> m p r c", m=NFT, p=128)
            for mf in range(NFT):
                dma_cast(
                    out=kt_all[mf][:].rearrange("p (r c) -> p r c",
                                                r=NGT // 2),
                    in_=kg[mf][:, 0:NGT // 2, :])
            for tt8 in range(NGT):
                r, tt = tt8 // 2, tt8 % 2
                vsrc = gath[r][E * TO:2 * E * TO].rearrange(
                    "(t p e) -> t p e", t=NTT, p=128)[tt].rearrange(
                    "p (h e) -> p h e", h=H)
                dma_cast(
                    out=vt_all[tt8].rearrange("p (h e) -> p h e",
                                              h=H)[:, :, 0:64],
                    in_=vsrc[:])

        if STAGE == 2:
            bail(klocal)
            return

        # ---- attention ----
        # upair quads: heads 0-3 in upA, 4-7 in upB; numerator rows 0-63,
        # denominator (ones-column of V) at row 64. First local-AV write per
        # bank uses start=True to clear stale has_written bits.
        upA = pstile(f"upA{l}")
        upB = pstile(f"upB{l}")
        up = [upA, upB]

        def scol(hh):
            # score-column layout: row-paired heads (hh even at partitions
            # 0-63, hh odd at 64-127) run CONCURRENTLY on the PE, so they
            # must drain into different PSUM banks
            return (hh % 2) * 512 + (hh // 2) * 256

        def attend(keysrc_fn, vsrc, nloc, mask_mul, bias_kt):
            """one 128-key block x 8 heads: scores -> exp -> AV"""
            for grp in range(2):        # head groups: 0-3 / 4-7
                sc = pstile()
                for hh in range(4):
                    h = grp * 4 + hh
                    lhsT = keysrc_fn(h)
                    nc.tensor.matmul(
                        sc[:, scol(hh):scol(hh) + 256],
                        lhsT,
                        qt[(h % 2) * 64:(h % 2) * 64 + 64,
                           (h // 2) * 256:(h // 2) * 256 + 256],
                        start=True, stop=True)
                es = p.es.tile([128, 1024], bf16, tag="es",
                               name=f"es{l}_{nloc}_{grp}")
                if bias_kt is None:
                    nc.scalar.activation(es[:], sc[:], AF.Exp)
                else:
                    nc.scalar.activation(
                        es[:], sc[:], AF.Exp,
                        bias=maskb_t[:, bias_kt:bias_kt + 1])
                if mask_mul is not None:
                    # mask is head-independent: same [128, 4*TO] tile for
                    # both head groups
                    nc.vector.tensor_mul(es[:], es[:], mask_mul[:])
                for hh in range(4):
                    h = grp * 4 + hh
                    nc.tensor.matmul(
                        up[grp][0:65, hh * 256:(hh + 1) * 256],
                        vsrc[:, h * 65:(h + 1) * 65],
                        es[:, scol(hh):scol(hh) + 256],
                        start=(nloc == 0 and hh % 2 == 0),
                        stop=(nloc == NTT + NGT - 1 and hh % 2 == 1),
                        skip_group_check=True)

        nloc = 0
        for lt in range(NTT):
            attend(lambda h, lt=lt: klocal[
                       (h % 2) * 64:(h % 2) * 64 + 64,
                       (h // 2) * 256 + lt * 128:(h // 2) * 256 + lt * 128 + 128],
                   vtloc[lt], nloc, moR[lt], None)
            nloc += 1
        for kti in range(NGT):
            attend(lambda h, kti=kti: kt_all[h // 2][
                       (h % 2) * 64:(h % 2) * 64 + 64,
                       kti * 128:(kti + 1) * 128],
                   vt_all[kti], nloc, None, kti)
            nloc += 1

        if STAGE == 3:
            cp3 = p.act.tile([128, 1024], bf16, tag="qt", name="cp3", bufs=2)
            nc.vector.tensor_copy(cp3[0:64, :], upA[0:64, :])
            nc.vector.tensor_copy(cp3[64:128, :], upB[0:64, :])
            bail(cp3)
            return

        # ---- normalize heads -> conc^T [E, TO] bf16 ----
        rec = p.stat.tile([1, 2048], f32, tag="rec", bufs=1)
        if os.environ.get("K_NO_RECIP_APPROX"):
            nc.vector.reciprocal(rec[:, 0:1024], upA[64:65, :])
            nc.vector.reciprocal(rec[:, 1024:2048], upB[64:65, :])
        else:
            # reciprocal_approx_fast misreads PSUM operands on HW: stage
            # the denominator rows through SBUF first
            den = p.stat.tile([1, 2048], f32, tag="den", bufs=1)
            nc.vector.tensor_copy(den[:, 0:1024], upA[64:65, :])
            nc.vector.tensor_copy(den[:, 1024:2048], upB[64:65, :])
            nc.vector.reciprocal_approx_fast(rec[:], den[:])
        rbq = [pstile(f"rbq{l}a"), pstile(f"rbq{l}b")]
        for grp in range(2):
            for hh in range(4):
                nc.tensor.matmul(
                    rbq[grp][0:64, hh * 256:(hh + 1) * 256],
                    ones_r[0:1, 0:64],
                    rec[:, grp * 1024 + hh * 256: grp * 1024 + (hh + 1) * 256],
                    start=True, stop=True)
        rbs = p.stat.tile([64, 2048], bf16, tag="rbs", bufs=1)
        nc.vector.tensor_copy(rbs[:, 0:1024], rbq[0][0:64, :])
        nc.vector.tensor_copy(rbs[:, 1024:2048], rbq[1][0:64, :])
        conc = p.act.tile([128, NFT * 256], bf16, tag="conc", bufs=2)
        for h in range(H):
            grp, hh = h // 4, h % 4
            nc.vector.tensor_mul(
                conc[(h % 2) * 64:(h % 2) * 64 + 64,
                     (h // 2) * 256:(h // 2) * 256 + 256],
                up[grp][0:64, hh * 256:(hh + 1) * 256],
                rbs[:, grp * 1024 + hh * 256: grp * 1024 + (hh + 1) * 256])

        w2a = p.wff.tile([128, 4096], bf16, tag="wff")
        nc.sync.dma_start(
            w2a[:].rearrange("p (k n) -> p k n", k=8),
            io["w2"][l][0:1024, :].rearrange("(k p) n -> p k n", p=128)[:])
        w2b = p.wff.tile([128, 4096], bf16, tag="wff")
        nc.sync.dma_start(
            w2b[:].rearrange("p (k n) -> p k n", k=8),
            io["w2"][l][1024:2048, :].rearrange("(k p) n -> p k n", p=128)[:])

        # ---- mha^T + residual + LN1 ----
        psW = pstile(f"psW{l}")
        for mf in range(NFT):
            for kf in range(NFT):
                nc.tensor.matmul(
                    psW[:, mf * 256:(mf + 1) * 256],
                    wo_t[:, kf * 512 + mf * 128: kf * 512 + (mf + 1) * 128],
                    conc[:, kf * 256:(kf + 1) * 256],
                    start=(kf == 0), stop=(kf == NFT - 1))
        res1 = p.act.tile([128, NFT * 256], f32, tag="res", bufs=2)
        for mf in range(NFT):
            nc.vector.tensor_scalar_add(
                res1[:, mf * 256:(mf + 1) * 256],
                psW[:, mf * 256:(mf + 1) * 256],
                bo_t[:, l * NFT + mf: l * NFT + mf + 1])
        nc.vector.tensor_add(res1[:], res1[:], ht[:])

        if STAGE == 4:
            bail(conc)
            return

        ln1f, ln1b = _layernorm(nc, p, ones_f, ones_r, res1,
                                g1_t, be1_t, l, "ln1", pstile)

        if STAGE == 5:
            bail(ln1b)
            return

        # ---- FFN ----
        a1t = []
        for ag in range(2):
            a1 = p.act.tile([128, 8 * 256], bf16, tag="a1", bufs=2)
            a1t.append(a1)
            for half in range(2):
                psA = pstile()
                for kf in range(NFT):
                    for m4 in range(4):
                        mt = ag * 8 + half * 4 + m4
                        wsrc = w1a if mt < 8 else w1b
                        moff = mt % 8
                        nc.tensor.matmul(
                            psA[:, m4 * 256:(m4 + 1) * 256],
                            wsrc[:, kf * 1024 + moff * 128: kf * 1024 + (moff + 1) * 128],
                            ln1b[:, kf * 256:(kf + 1) * 256],
                            start=(kf == 0 and m4 % 2 == 0),
                            stop=(kf == NFT - 1 and m4 % 2 == 1),
                            skip_group_check=True)
                for m4 in range(4):
                    mt = ag * 8 + half * 4 + m4
                    dst = a1[:, (half * 4 + m4) * 256:(half * 4 + m4 + 1) * 256]
                    src = psA[:, m4 * 256:(m4 + 1) * 256]
                    bia = b1_t[:, l * NMT + mt: l * NMT + mt + 1]
                    if m4 % 2 == 0:
                        nc.vector.tensor_scalar(
                            dst, src, bia, 0.0, ALU.add, ALU.max)
                    else:
                        nc.scalar.activation(dst, src, AF.Relu, bias=bia)

        psR = pstile(f"psR{l}")
        for kt2 in range(NMT):
            wsrc = w2a if kt2 < 8 else w2b
            koff = kt2 % 8
            for mf in range(NFT):
                nc.tensor.matmul(
                    psR[:, mf * 256:(mf + 1) * 256],
                    wsrc[:, koff * 512 + mf * 128: koff * 512 + (mf + 1) * 128],
                    a1t[kt2 // 8][:, (kt2 % 8) * 256:(kt2 % 8 + 1) * 256],
                    start=(kt2 == 0 and mf % 2 == 0),
                    stop=(kt2 == NMT - 1 and mf % 2 == 1),
                    skip_group_check=True)
        res2 = p.act.tile([128, NFT * 256], f32, tag="res", bufs=2)
        for mf in range(NFT):
            nc.vector.tensor_scalar_add(
                res2[:, mf * 256:(mf + 1) * 256],
                psR[:, mf * 256:(mf + 1) * 256],
                b2_t[:, l * NFT + mf: l * NFT + mf + 1])
        nc.vector.tensor_add(res2[:], res2[:], ln1f[:])

        ht, htb = _layernorm(nc, p, ones_f, ones_r, res2,
                             g2_t, be2_t, l, "ln2", pstile)

    if SKIP_VOCAB:
        nc.sync.dma_start(io["out"][0][:, 0:NFT * TO], htb[:])
        return
    if NLAYERS < 2:
        for kf in range(NFT):
            nc.sync.dma_start(wout_sb[kf][:], io["wout"][kf])

    # ================= vocab-sharded projection =================
    # AllGather the final hidden state (bf16, feature-major) across all 8
    # cores, then project all 2048 tokens against this core's 4096-padded
    # vocab shard with Wout already resident in SBUF.
    contribH = p.dram.tile([E * TO], bf16, tag="contribH")
    nc.sync.dma_start(
        contribH[:].rearrange("(m p c) -> p m c", p=128, m=NFT),
        htb[:].rearrange("p (m c) -> p m c", m=NFT))
    gathH = p.dram.tile([NC, E * TO], bf16, tag="gathH",
                        addr_space="Local" if COLLFREE else "Shared")
    if COLLFREE:
        for rr in range(NC):
            nc.sync.dma_start(gathH[rr], contribH[:])
    else:
        nc.gpsimd.collective_compute(
            "AllGather", ALU.bypass,
            replica_groups=[[0, 1, 2, 3, 4, 5, 6, 7]],
            ins=[contribH[:]], outs=[gathH[:]])
    def project(tb_out, stat_fn):
        """project one 128-token block against the full vocab shard"""
        duos = [pstile() for _ in range(4)]
        for kf in range(NFT):
            for vc in range(VSP // 512):
                nc.tensor.matmul(
                    duos[vc // 2][:, (vc % 2) * 512:(vc % 2 + 1) * 512],
                    stat_fn(kf),
                    wout_sb[kf][:, vc * 512:(vc + 1) * 512],
                    start=(kf == 0), stop=(kf == NFT - 1))
        for half in range(2):
            ob = p.out.tile([128, VSP // 2], bf16, tag="ob")
            for v2 in range(4):
                vc = half * 4 + v2
                dst = ob[:, v2 * 512:(v2 + 1) * 512]
                src = duos[vc // 2][:, (vc % 2) * 512:(vc % 2 + 1) * 512]
                if vc % 2 == 0:
                    nc.vector.tensor_copy(dst, src)
                else:
                    nc.scalar.copy(dst, src)
            nc.sync.dma_start(
                io["out"][tb_out][:, half * (VSP // 2):(half + 1) * (VSP // 2)],
                ob[:])

    # own token blocks first, straight from local htb -- overlaps the
    # AllGather; the host uses slots NTB..NTB+1 for this core's rows
    for h2 in range(NTT):
        project(NTB + h2,
                lambda kf, h2=h2: htb[:, kf * 256 + h2 * 128:
                                      kf * 256 + h2 * 128 + 128])

    htg = []
    hgv = gathH.rearrange("r (m p c) -> m p r c", m=NFT, p=128)
    for kf in range(NFT):
        t = p.hg.tile([128, NC * TO], bf16, tag="hg", name=f"htg{kf}")
        nc.sync.dma_start(t[:].rearrange("p (r c) -> p r c", r=NC), hgv[kf])
        htg.append(t)

    for tb in range(NTB):
        project(tb, lambda kf, tb=tb: htg[kf][:, tb * 128:(tb + 1) * 128])


def _layernorm(nc, p, ones_f, ones_r, res, g_t, b_t, l, name, pstile):
    """Feature-major layernorm over a [128, NFT*TO] f32 quad -> (f32, bf16).

    Statistics are computed from a bf16 copy so the partition-sum matmuls
    stream bf16 (fp32-moving matmuls are 4x slower on the PE). Both LN1 and
    LN2 outputs share the lnf/lnb tags (bufs=3) to bound SBUF."""
    resb = p.act.tile([128, NFT * 256], bf16, tag="resb", bufs=2)
    nc.vector.tensor_copy(resb[:], res[:])
    sq = p.act.tile([128, NFT * 256], bf16, tag="sq", bufs=2)
    nc.vector.tensor_mul(sq[:], resb[:], resb[:])
    stats = pstile(f"stats_{name}{l}")
    for kf in range(NFT):
        nc.tensor.matmul(stats[0:1, 0:256], ones_f[:, :],
                         resb[:, kf * 256:(kf + 1) * 256],
                         start=(kf == 0), stop=(kf == NFT - 1))
    for kf in range(NFT):
        nc.tensor.matmul(stats[32:33, 0:256], ones_f[:, :],
                         sq[:, kf * 256:(kf + 1) * 256],
                         start=(kf == 0), stop=(kf == NFT - 1))
    sv = p.stat.tile([1, 4 * TO], f32, tag="stat", bufs=1)
    mu = sv[:, 0:TO]
    musq = sv[:, TO:2 * TO]
    var = sv[:, 2 * TO:3 * TO]
    std = sv[:, TO:2 * TO]          # reuses musq slot (musq dead)
    rstd = sv[:, 3 * TO:4 * TO]
    murstd = sv[:, 2 * TO:3 * TO]   # reuses var slot (var dead)
    nc.vector.tensor_scalar_mul(mu, stats[0:1, 0:256], 1.0 / E)
    nc.vector.tensor_mul(musq, mu, mu)
    nc.vector.tensor_scalar(var, stats[32:33, 0:256], 1.0 / E, EPS,
                            ALU.mult, ALU.add)
    nc.vector.tensor_sub(var, var, musq)
    # rstd = exp(-0.5*ln(var)); ln+exp live in the single pinned table set
    nc.scalar.activation(std, var, AF.Ln)
    nc.scalar.activation(rstd, std, AF.Exp, scale=-0.5)
    nc.vector.tensor_mul(murstd, mu, rstd)
    # rb/mb broadcasts into bank 1 of the stats psum tile (cols 512..1023)
    rb = stats[:, 512:768]
    mb = stats[:, 768:1024]
    nc.tensor.matmul(rb, ones_r[:, :], rstd, start=True, stop=True)
    nc.tensor.matmul(mb, ones_r[:, :], murstd, start=True, stop=True)
    outf = p.act.tile([128, NFT * 256], f32, tag="lnf", bufs=3,
                      name=f"{name}f{l}")
    outb = p.act.tile([128, NFT * 256], bf16, tag="lnb", bufs=3,
                      name=f"{name}b{l}")
    for kf in range(NFT):
        t = outf[:, kf * 256:(kf + 1) * 256]
        nc.vector.tensor_mul(t, res[:, kf * 256:(kf + 1) * 256], rb)
        nc.vector.tensor_sub(t, t, mb)
        nc.vector.tensor_scalar(
            t, t,
            g_t[:, l * NFT + kf: l * NFT + kf + 1],
            b_t[:, l * NFT + kf: l * NFT + kf + 1],
            ALU.mult, ALU.add)
        # per-chunk bf16 cast so downstream matmuls start on chunk 0
        # while later chunks are still being normalized
        nc.vector.tensor_copy(outb[:, kf * 256:(kf + 1) * 256], t)
    return outf, outb


def _prep_inputs(x, tok_emb, pos_emb, Wq, bq, Wk, bk, Wv, bv, Wo, bo,
                 W1, b1, W2, b2, ln1_g, ln1_b, ln2_g, ln2_b, Wout, bout):
    """Host-side sharding: returns in_maps for the 8 cores."""
    x = np.asarray(x)
    h0 = np.asarray(tok_emb)[x] + np.asarray(pos_emb)[None, :, :]   # [B,S,E] f32
    h0t = np.ascontiguousarray(np.transpose(h0, (0, 2, 1)))          # [B,E,S]

    scale = 1.0 / np.sqrt(D)
    wq_h = (np.transpose(np.asarray(Wq), (0, 2, 1, 3)).reshape(L, E, H * D)
            * scale).astype(BF16)
    wk_h = np.transpose(np.asarray(Wk), (0, 2, 1, 3)).reshape(L, E, H * D).astype(BF16)
    wv_h = np.transpose(np.asarray(Wv), (0, 2, 1, 3)).reshape(L, E, H * D).astype(BF16)
    wo_h = np.asarray(Wo).astype(BF16)
    w1_h = np.asarray(W1).astype(BF16)
    w2_h = np.asarray(W2).astype(BF16)
    bq_h = (np.asarray(bq).reshape(L, H * D) * scale).astype(np.float32)
    bk_h = np.asarray(bk).reshape(L, H * D).astype(np.float32)
    bv_c = np.asarray(bv).reshape(L, H * D).astype(np.float32)
    bo_eff = (np.asarray(bo) + np.einsum("lc,lce->le", bv_c, np.asarray(Wo))
              ).astype(np.float32)
    wout_np = np.zeros((NFT, 128, NC, VSP), dtype=BF16)
    wfull = np.asarray(Wout).astype(BF16).reshape(NFT, 128, V)
    for c in range(NC):
        wout_np[:, :, c, :VS] = wfull[:, :, c * VS:(c + 1) * VS]

    # pack all [L, E]-style bias/scale vectors (+ the per-core key-block
    # mask appended later) into one [128, 200] f32 tensor: col l*n+k holds
    # arr[l, k*128+p] for partition p
    def pk(arr, n=NFT):
        return np.ascontiguousarray(
            np.asarray(arr, dtype=np.float32)
            .reshape(L, n, 128).transpose(2, 0, 1).reshape(128, L * n))

    bvec_c = np.concatenate([
        pk(bq_h), pk(bk_h), pk(bo_eff), pk(b2),
        pk(ln1_g), pk(ln1_b), pk(ln2_g), pk(ln2_b),
        pk(b1, n=NMT)], axis=1)

    common = dict(
        wq=wq_h, wk=wk_h, wv=wv_h, wo=wo_h, w1=w1_h, w2=w2_h,
    )

    in_maps = []
    for c in range(NC):
        b, j = c // G, c % G
        # own-block causal mask, replicated across 4 heads (both head
        # groups reuse the same tile): [NTT*128, 4*TO]
        qpos = j * TO + np.arange(TO)[None, :]
        moR = np.zeros((NTT * 128, 4 * TO), dtype=BF16)
        for lt in range(NTT):
            kpos = j * TO + lt * 128 + np.arange(128)[:, None]
            m = (kpos <= qpos).astype(BF16)          # [128, TO]
            moR[lt * 128:(lt + 1) * 128] = np.tile(m, (1, 4))
        # gathered-path visibility per 128-key block: fully visible (0.0)
        # only strictly below this core's own rows; own rows come from the
        # local pass, everything else exp(-30)-masked
        maskb = np.full((128, NKT), -30.0, np.float32)
        maskb[:, :2 * j] = 0.0
        in_maps.append(dict(
            common,
            h0t_full=h0t[b].astype(BF16),
            h0t_own=np.ascontiguousarray(
                h0t[b][:, j * TO:(j + 1) * TO]).astype(np.float32),
            bvec=np.ascontiguousarray(
                np.concatenate([bvec_c, maskb], axis=1)),
            maskOwnR=moR,
            wout=np.ascontiguousarray(wout_np[:, :, c, :]),
        ))
    return in_maps


def _finish_output(res, bout):
    bout = np.asarray(bout, dtype=np.float32)
    logits = np.empty((B, S, V), dtype=np.float32)
    for c in range(NC):
        o = np.asarray(res.results[c]["out"], dtype=np.float32)  # [NTB+2,...]
        for tb in range(NTB):
            r = tb // 2
            bb, j = r // G, r % G
            t0 = j * TO + (tb % 2) * 128
            # own token blocks come from the early (pre-AllGather) slots
            src = NTB + (tb % 2) if r == c else tb
            logits[bb, t0:t0 + 128, c * VS:(c + 1) * VS] = o[src][:, :VS]
    logits += bout[None, None, :]
    return logits


def kernel(**inputs):
    if "nc" not in _cache:
        _cache["nc"] = build_nc()
    nc = _cache["nc"]
    inputs = {k: np.asarray(v) for k, v in inputs.items()}
    in_maps = _prep_inputs(**inputs)
    res = run_bass_kernel_spmd(nc, in_maps, list(range(NC)))
    return _finish_output(res, inputs["bout"])


# revision 98
# speedup vs baseline: 1.4406x; 1.0537x over previous
"""GPT forward pass on 8 Trainium2 NeuronCores.

Sharding: cores 0-3 handle batch 0, cores 4-7 batch 1; within each 4-core
group the 1024 tokens are sequence-sharded 256/core. Activations are kept
feature-major (transposed) on chip. Per layer each core computes Q/K/V for
its own tokens; K and V are AllGathered in ONE fused bf16 collective within
the 4-core group (layer 0 computes full K/V from h0 directly, no
collective). Attention exponentials are batched 4-heads-at-a-time over
2-bank PSUM tiles. The final vocab projection is VOCAB-sharded: after an
8-core AllGather of the final hidden state, each core projects all 2048
tokens against its own 4000 vocab columns with Wout fully prefetched in
SBUF; the output bias is added on the host.
"""

import os
import sys

for _p in ("/opt/trn_rl_repo", "/root/.axon_site/_ro/trn_rl_repo"):
    if os.path.isdir(_p) and _p not in sys.path:
        sys.path.insert(0, _p)

import ml_dtypes
import numpy as np

import concourse.bass as bass
import concourse.mybir as mybir
import concourse.tile as tile
from concourse import bacc
from concourse.bass_utils import run_bass_kernel_spmd

BF16 = ml_dtypes.bfloat16
f32 = mybir.dt.float32
bf16 = mybir.dt.bfloat16
AF = mybir.ActivationFunctionType
ALU = mybir.AluOpType

V, S, E, H, D, L = 32000, 1024, 512, 8, 64, 4
FF = 4 * E
B = 2
NC = 8
G = 4            # cores per batch group
TO = S // G      # tokens owned per core (256)
EPS = 1e-5
NKT = S // 128   # key tiles (8)
NGT = NKT - 2    # gathered key tiles actually attended (6): tiles 6,7 are
                 # above every core's own rows -> always exp(-30)-masked
NFT = E // 128   # feature tiles (4)
NTT = TO // 128  # own-token tiles (2)
NMT = FF // 128  # FFN hidden tiles (16)
VS = V // NC     # vocab columns per core (4000)
VSP = 4096       # padded vocab shard
NTB = (B * S) // 128   # token blocks in vocab phase (16)

_cache = {}
COLLFREE = False


def _pin_act_tables():
    """Force every activation function this kernel uses into the single
    `natural_log_exp_and_others` table set so the compiler never emits a
    mid-kernel ACT_TABLE_LOAD swap (each swap costs ~2.7us on ScalarE).
    The set genuinely contains ln/exp/square/relu/copy/identity."""
    import concourse.hw_specs as hw_specs

    if getattr(hw_specs, "_act_tables_pinned", False):
        return
    orig = hw_specs.get_activation_tables

    import functools

    @functools.cache
    def patched(module_arch):
        tabs = {k: set(v) for k, v in orig(module_arch).items()}
        combo = "natural_log_exp_and_others"
        if combo not in tabs:
            return tabs
        keep = tabs[combo]
        for name, fns in tabs.items():
            if name != combo:
                fns -= keep
        return tabs

    hw_specs.get_activation_tables = patched
    bacc.get_activation_tables = patched
    hw_specs._act_tables_pinned = True


def build_nc(trace=False, rep=1):
    if not os.environ.get("K_NO_ACTPIN"):
        _pin_act_tables()
    nc = bacc.Bacc("TRN2", target_bir_lowering=False, debug=False,
                   num_devices=1 if COLLFREE else NC)

    def din(name, shape, dt):
        return nc.dram_tensor(name, shape, dt, kind="ExternalInput").ap()

    io = dict(
        h0t_full=din("h0t_full", [E, S], bf16),
        h0t_own=din("h0t_own", [E, TO], f32),
        bvec=din("bvec", [128, 8 * L * NFT + L * NMT + NKT], f32),
        maskOwnR=din("maskOwnR", [NTT * 128, 4 * TO], bf16),
        wq=din("wq", [L, E, H * D], bf16),
        wk=din("wk", [L, E, H * D], bf16),
        wv=din("wv", [L, E, H * D], bf16),
        wo=din("wo", [L, H * D, E], bf16),
        w1=din("w1", [L, E, FF], bf16),
        w2=din("w2", [L, FF, E], bf16),
        wout=din("wout", [NFT, 128, VSP], bf16),
        out=nc.dram_tensor("out", [NTB + NTT, 128, VSP], bf16,
                           kind="ExternalOutput").ap(),
    )

    with tile.TileContext(nc) as tc:
        _body(nc, tc, io, rep=rep)
    nc.compile()
    return nc


class P:
    """pool handles"""


def _body(nc, tc, io, rep=1):
    from contextlib import ExitStack

    ctx = ExitStack()
    with ctx:
        p = P()
        p.const = ctx.enter_context(tc.tile_pool(name="const", bufs=1))
        p.w512 = ctx.enter_context(tc.tile_pool(name="w512", bufs=4))
        p.wff = ctx.enter_context(tc.tile_pool(name="wff", bufs=3))
        p.kv = ctx.enter_context(tc.tile_pool(name="kv", bufs=1))
        p.hg = ctx.enter_context(tc.tile_pool(name="hg", bufs=4))
        p.act = ctx.enter_context(tc.tile_pool(name="act", bufs=1))
        p.es = ctx.enter_context(tc.tile_pool(name="esp", bufs=2))
        p.stat = ctx.enter_context(tc.tile_pool(name="stat", bufs=2))
        p.out = ctx.enter_context(tc.tile_pool(name="pout", bufs=2))
        p.ps = ctx.enter_context(tc.tile_pool(name="ps", bufs=4, space="PSUM"))
        p.dram = ctx.enter_context(tc.tile_pool(name="dram", bufs=2, space="DRAM"))

        _psn = [0]

        def pstile(name=None):
            if name is None:
                _psn[0] += 1
                name = f"ps{_psn[0]}"
            return p.ps.tile([128, 1024], f32, tag="ps", name=name)

        # ---- constants ----
        ones_f = p.const.tile([128, 1], f32, tag="ones_f")
        nc.vector.memset(ones_f[:], 1.0)
        ones_fb = p.const.tile([128, 1], bf16, tag="ones_fb")
        nc.vector.memset(ones_fb[:], 1.0)
        p.ones_fb = ones_fb
        ones_r = p.const.tile([1, 128], f32, tag="ones_r")
        nc.vector.memset(ones_r[:], 1.0)

        # all per-feature bias/scale vectors + block mask, packed host-side
        # into one contiguous [128, 200] f32 tensor -> a single fast DMA
        NB = 8 * L * NFT + L * NMT + NKT
        bvec_t = p.const.tile([128, NB], f32, tag="bvec")
        nc.sync.dma_start(bvec_t[:], io["bvec"][:, :])
        _off = [0]

        def bslice(n):
            t = bvec_t[:, _off[0]:_off[0] + n]
            _off[0] += n
            return t

        bq_t = bslice(L * NFT)
        bk_t = bslice(L * NFT)
        bo_t = bslice(L * NFT)
        b2_t = bslice(L * NFT)
        g1_t = bslice(L * NFT)
        be1_t = bslice(L * NFT)
        g2_t = bslice(L * NFT)
        be2_t = bslice(L * NFT)
        b1_t = bslice(L * NMT)
        maskb_t = bslice(NKT)
        # moR/h0f DMAs are issued from inside layer 0, after the layer-0
        # weight loads, to get the PE started as early as possible
        moR = []
        for lt in range(NTT):
            m = p.const.tile([128, 4 * TO], bf16, tag=f"moR{lt}")
            moR.append(m)

        # Wout shard (fp8, one [128, NFT*VSP] tile so DoubleRow matmuls can
        # slice kf-pairs): resident in SBUF for the whole kernel; the DMAs
        # are issued from inside layer 1 so layer-0 weights load first
        wout_sb = p.const.tile([128, NFT * VSP], bf16, tag="wout")

        # prime the ncfw collective path for the 4-rank groups during the
        # startup window so layer 1's first real AllGather starts warm
        if not COLLFREE:
            warm_sb = p.const.tile([128, 16], bf16, tag="warm_sb")
            nc.vector.memset(warm_sb[:], 0.0)
            warm_in = p.dram.tile([2048], bf16, tag="warm_in", bufs=1)
            nc.sync.dma_start(
                warm_in[:].rearrange("(p c) -> p c", p=128), warm_sb[:])
            warm_out = p.dram.tile([G, 2048], bf16, tag="warm_out", bufs=1)
            nc.gpsimd.collective_compute(
                "AllGather", ALU.bypass,
                replica_groups=[[0, 1, 2, 3], [4, 5, 6, 7]],
                ins=[warm_in[:]], outs=[warm_out[:]])

        # full h0 (bf16, feature-major) for layer-0 K/V
        h0f = []
        for kf in range(NFT):
            t = p.hg.tile([128, S], bf16, tag="hg", name=f"h0f{kf}")
            h0f.append(t)

        # initial hidden state (own tokens, f32 quad [128, NFT*TO]);
        # shares tags with the per-layer LN2 outputs
        ht = p.act.tile([128, NFT * TO], f32, tag="lnf", bufs=3)
        nc.sync.dma_start(
            ht[:].rearrange("p (k c) -> p k c", k=NFT),
            io["h0t_own"].rearrange("(k p) c -> p k c", p=128)[:])
        htb = p.act.tile([128, NFT * TO], bf16, tag="lnb", bufs=3)
        nc.vector.tensor_copy(htb[:], ht[:])

        # persistent K^T / V tiles (gathered, key tiles 0..NGT-1 only);
        # ones columns set once
        kt_all = []
        for mf in range(NFT):
            kt = p.kv.tile([128, NGT * 128], bf16, tag=f"kt{mf}")
            kt_all.append(kt)
        vt_all = []
        for tt in range(NGT):
            vt = p.kv.tile([128, H * 65], bf16, tag=f"vt{tt}")
            nc.vector.memset(
                vt.rearrange("p (h e) -> p h e", h=H)[:, :, 64:65], 1.0)
            vt_all.append(vt)
        vtloc = []
        for lt in range(NTT):
            vl = p.kv.tile([128, H * 65], bf16, tag=f"vtloc{lt}")
            nc.vector.memset(
                vl.rearrange("p (h e) -> p h e", h=H)[:, :, 64:65], 1.0)
            vtloc.append(vl)

        def load_w512(ap, l):
            t = p.w512.tile([128, NFT * 512], bf16, tag="w512")
            nc.sync.dma_start(t[:].rearrange("p (k n) -> p k n", k=NFT),
                              ap[l].rearrange("(k p) n -> p k n", p=128)[:])
            return t

        env = dict(locals())
        for _rep in range(rep):
            _compute(nc, tc, io, p, env)


def _compute(nc, tc, io, p, env):
    pstile = env["pstile"]
    ones_f = env["ones_f"]; ones_r = env["ones_r"]
    bq_t = env["bq_t"]; bk_t = env["bk_t"]; bo_t = env["bo_t"]; b2_t = env["b2_t"]
    g1_t = env["g1_t"]; be1_t = env["be1_t"]; g2_t = env["g2_t"]; be2_t = env["be2_t"]
    b1_t = env["b1_t"]; maskb_t = env["maskb_t"]; moR = env["moR"]
    wout_sb = env["wout_sb"]; h0f = env["h0f"]
    kt_all = env["kt_all"]; vt_all = env["vt_all"]; vtloc = env["vtloc"]
    load_w512 = env["load_w512"]
    ht = env["ht"]; htb = env["htb"]

    NLAYERS = int(os.environ.get("K_NL", L))
    SKIP_VOCAB = bool(os.environ.get("K_NOVOCAB"))
    STAGE = int(os.environ.get("K_STAGE", 99))
    AG_BF16 = bool(os.environ.get("K_AG_BF16"))

    def bail(t):
        nc.sync.dma_start(io["out"][0][:, 0:t.shape[-1]], t[:])

    # ================= transformer layers =================
    for l in range(NLAYERS):
        wq_t = load_w512(io["wq"], l)
        wk_t = load_w512(io["wk"], l)
        wv_t = load_w512(io["wv"], l)
        if l == 0:
            for kf in range(NFT):
                nc.sync.dma_start(h0f[kf][:],
                                  io["h0t_full"][kf * 128:(kf + 1) * 128, :])
            for lt in range(NTT):
                nc.sync.dma_start(
                    moR[lt][:], io["maskOwnR"][lt * 128:(lt + 1) * 128, :])
        if l == 1:
            # prefetch the SBUF-resident Wout shard now that layer-0
            # weights are already in flight
            for kf in range(NFT):
                nc.sync.dma_start(
                    wout_sb[:, kf * VSP:(kf + 1) * VSP], io["wout"][kf])

        hb = htb  # bf16 activations of this layer's input

        # ---- K/V for own tokens (feeds local pass; l>0 also the AG) ----
        # kf-outer so matmuls start as soon as hb chunk 0 exists; only the
        # bank-opening matmul (even mf, kf 0) uses start=True -- a second
        # start=True in the same bank would clear the first chunk's
        # has_written bits mid-accumulation
        psK = pstile(f"psK{l}")
        for kf in range(NFT):
            for mf in range(NFT):
                nc.tensor.matmul(
                    psK[:, mf * 256:(mf + 1) * 256],
                    wk_t[:, kf * 512 + mf * 128: kf * 512 + (mf + 1) * 128],
                    hb[:, kf * 256:(kf + 1) * 256],
                    start=(kf == 0 and mf % 2 == 0),
                    stop=(kf == NFT - 1 and mf % 2 == 1),
                    skip_group_check=True)
        klocal = p.act.tile([128, NFT * 256], bf16, tag="klocal", bufs=2)
        for mf in range(NFT):
            nc.vector.tensor_scalar_add(
                klocal[:, mf * 256:(mf + 1) * 256],
                psK[:, mf * 256:(mf + 1) * 256],
                bk_t[:, l * NFT + mf: l * NFT + mf + 1])

        psV = pstile(f"psV{l}")
        for kf in range(NFT):
            for tt in range(NTT):
                nc.tensor.matmul(
                    psV[:, tt * 512:(tt + 1) * 512],
                    hb[:, kf * 256 + tt * 128: kf * 256 + tt * 128 + 128],
                    wv_t[:, kf * 512:(kf + 1) * 512],
                    start=(kf == 0), stop=(kf == NFT - 1))
        vlocal = p.act.tile([128, NTT * 512], bf16, tag="vlocal", bufs=2)
        for tt in range(NTT):
            nc.vector.tensor_copy(
                vlocal[:, tt * 512:(tt + 1) * 512],
                psV[:, tt * 512:(tt + 1) * 512])
            nc.vector.tensor_copy(
                vtloc[tt].rearrange("p (h e) -> p h e", h=H)[:, :, 0:64],
                psV[:, tt * 512:(tt + 1) * 512]
                .rearrange("p (h e) -> p h e", h=H)[:, :, :])

        if l > 0:
            # K/V travel the AllGather in fp8e4m3 (half the wire bytes of
            # bf16 -- the 4-rank mesh AG is bandwidth-limited); the SWDGE
            # DMAs cast on the way out and back in
            wdt = bf16 if AG_BF16 else mybir.dt.float8e4
            dma_cast = nc.sync.dma_start if AG_BF16 else nc.gpsimd.dma_start
            contrib = p.dram.tile([2 * E * TO], wdt, tag="contrib")
            dma_cast(
                out=contrib[0:E * TO].rearrange("(m p c) -> p m c",
                                                p=128, m=NFT),
                in_=klocal[:].rearrange("p (m c) -> p m c", m=NFT))
            dma_cast(
                out=contrib[E * TO:2 * E * TO].rearrange(
                    "(t p e) -> p t e", p=128, t=NTT),
                in_=vlocal[:].rearrange("p (t e) -> p t e", t=NTT))
            gath = p.dram.tile([G, 2 * E * TO], wdt, tag="gath")
            if COLLFREE:
                for rr in range(G):
                    nc.sync.dma_start(gath[rr], contrib[:])
            else:
                nc.gpsimd.collective_compute(
                    "AllGather", ALU.bypass,
                    replica_groups=[[0, 1, 2, 3], [4, 5, 6, 7]],
                    ins=[contrib[:]], outs=[gath[:]])

        # ---- Q^T (1/sqrt(D) folded into wq/bq on host) ----
        psQ = pstile(f"psQ{l}")
        for kf in range(NFT):
            for mf in range(NFT):
                nc.tensor.matmul(
                    psQ[:, mf * 256:(mf + 1) * 256],
                    wq_t[:, kf * 512 + mf * 128: kf * 512 + (mf + 1) * 128],
                    hb[:, kf * 256:(kf + 1) * 256],
                    start=(kf == 0 and mf % 2 == 0),
                    stop=(kf == NFT - 1 and mf % 2 == 1),
                    skip_group_check=True)
        qt = p.act.tile([128, NFT * 256], bf16, tag="qt", bufs=2)
        for mf in range(NFT):
            nc.vector.tensor_scalar_add(
                qt[:, mf * 256:(mf + 1) * 256],
                psQ[:, mf * 256:(mf + 1) * 256],
                bq_t[:, l * NFT + mf: l * NFT + mf + 1])

        if STAGE == 1:
            bail(qt)
            return

        # prefetch remaining layer weights (overlaps attention)
        wo_t = load_w512(io["wo"], l)
        w1a = p.wff.tile([128, 4096], bf16, tag="wff")
        nc.sync.dma_start(
            w1a[:].rearrange("p (k n) -> p k n", k=4),
            io["w1"][l][:, 0:1024].rearrange("(k p) n -> p k n", p=128)[:])
        w1b = p.wff.tile([128, 4096], bf16, tag="wff")
        nc.sync.dma_start(
            w1b[:].rearrange("p (k n) -> p k n", k=4),
            io["w1"][l][:, 1024:2048].rearrange("(k p) n -> p k n", p=128)[:])

        # ---- gathered K/V: l==0 computes from full h0; l>0 loads AG ----
        if l == 0:
            for mf in range(NFT):
                psD = pstile(f"psD{mf}")
                for c2 in range(2):
                    cw = 512 if c2 == 0 else NGT * 128 - 512
                    for kf in range(NFT):
                        nc.tensor.matmul(
                            psD[:, c2 * 512:c2 * 512 + cw],
                            wk_t[:, kf * 512 + mf * 128: kf * 512 + (mf + 1) * 128],
                            h0f[kf][:, c2 * 512:c2 * 512 + cw],
                            start=(kf == 0), stop=(kf == NFT - 1))
                nc.vector.tensor_scalar_add(
                    kt_all[mf][:], psD[:, 0:NGT * 128],
                    bk_t[:, l * NFT + mf: l * NFT + mf + 1])
            for tp in range(NGT // 2):
                psE = pstile(f"psE{tp}")
                for half in range(2):
                    tt8 = tp * 2 + half
                    for kf in range(NFT):
                        nc.tensor.matmul(
                            psE[:, half * 512:(half + 1) * 512],
                            h0f[kf][:, tt8 * 128:(tt8 + 1) * 128],
                            wv_t[:, kf * 512:(kf + 1) * 512],
                            start=(kf == 0), stop=(kf == NFT - 1))
                for half in range(2):
                    tt8 = tp * 2 + half
                    nc.vector.tensor_copy(
                        vt_all[tt8].rearrange("p (h e) -> p h e", h=H)[:, :, 0:64],
                        psE[:, half * 512:(half + 1) * 512]
                        .rearrange("p (h e) -> p h e", h=H)[:, :, :])
        else:
            # only ranks 0..NGT/2-1 feed the attended gathered tiles
            dma_cast = nc.sync.dma_start if AG_BF16 else nc.gpsimd.dma_start
            kg = gath[:, 0:E * TO].rearrange(
                "r (m p c) -> m p r c", m=NFT, p=128)
            for mf in range(NFT):
                dma_cast(
                    out=kt_all[mf][:].rearrange("p (r c) -> p r c",
                                                r=NGT // 2),
                    in_=kg[mf][:, 0:NGT // 2, :])
            for tt8 in range(NGT):
                r, tt = tt8 // 2, tt8 % 2
                vsrc = gath[r][E * TO:2 * E * TO].rearrange(
                    "(t p e) -> t p e", t=NTT, p=128)[tt].rearrange(
                    "p (h e) -> p h e", h=H)
                dma_cast(
                    out=vt_all[tt8].rearrange("p (h e) -> p h e",
                                              h=H)[:, :, 0:64],
                    in_=vsrc[:])

        if STAGE == 2:
            bail(klocal)
            return

        # ---- attention ----
        # upair quads: heads 0-3 in upA, 4-7 in upB; numerator rows 0-63,
        # denominator (ones-column of V) at row 64. First local-AV write per
        # bank uses start=True to clear stale has_written bits.
        upA = pstile(f"upA{l}")
        upB = pstile(f"upB{l}")
        up = [upA, upB]

        def scol(hh):
            # score-column layout: row-paired heads (hh even at partitions
            # 0-63, hh odd at 64-127) run CONCURRENTLY on the PE, so they
            # must drain into different PSUM banks
            return (hh % 2) * 512 + (hh // 2) * 256

        def attend(keysrc_fn, vsrc, nloc, mask_mul, bias_kt):
            """one 128-key block x 8 heads: scores -> exp -> AV"""
            for grp in range(2):        # head groups: 0-3 / 4-7
                sc = pstile()
                for hh in range(4):
                    h = grp * 4 + hh
                    lhsT = keysrc_fn(h)
                    nc.tensor.matmul(
                        sc[:, scol(hh):scol(hh) + 256],
                        lhsT,
                        qt[(h % 2) * 64:(h % 2) * 64 + 64,
                           (h // 2) * 256:(h // 2) * 256 + 256],
                        start=True, stop=True)
                es = p.es.tile([128, 1024], bf16, tag="es",
                               name=f"es{l}_{nloc}_{grp}")
                if bias_kt is None:
                    nc.scalar.activation(es[:], sc[:], AF.Exp)
                else:
                    nc.scalar.activation(
                        es[:], sc[:], AF.Exp,
                        bias=maskb_t[:, bias_kt:bias_kt + 1])
                if mask_mul is not None:
                    # mask is head-independent: same [128, 4*TO] tile for
                    # both head groups
                    nc.vector.tensor_mul(es[:], es[:], mask_mul[:])
                for hh in range(4):
                    h = grp * 4 + hh
                    nc.tensor.matmul(
                        up[grp][0:65, hh * 256:(hh + 1) * 256],
                        vsrc[:, h * 65:(h + 1) * 65],
                        es[:, scol(hh):scol(hh) + 256],
                        start=(nloc == 0 and hh % 2 == 0),
                        stop=(nloc == NTT + NGT - 1 and hh % 2 == 1),
                        skip_group_check=True)

        nloc = 0
        for lt in range(NTT):
            attend(lambda h, lt=lt: klocal[
                       (h % 2) * 64:(h % 2) * 64 + 64,
                       (h // 2) * 256 + lt * 128:(h // 2) * 256 + lt * 128 + 128],
                   vtloc[lt], nloc, moR[lt], None)
            nloc += 1
        for kti in range(NGT):
            attend(lambda h, kti=kti: kt_all[h // 2][
                       (h % 2) * 64:(h % 2) * 64 + 64,
                       kti * 128:(kti + 1) * 128],
                   vt_all[kti], nloc, None, kti)
            nloc += 1

        if STAGE == 3:
            cp3 = p.act.tile([128, 1024], bf16, tag="qt", name="cp3", bufs=2)
            nc.vector.tensor_copy(cp3[0:64, :], upA[0:64, :])
            nc.vector.tensor_copy(cp3[64:128, :], upB[0:64, :])
            bail(cp3)
            return

        # ---- normalize heads -> conc^T [E, TO] bf16 ----
        rec = p.stat.tile([1, 2048], f32, tag="rec", bufs=1)
        if os.environ.get("K_NO_RECIP_APPROX"):
            nc.vector.reciprocal(rec[:, 0:1024], upA[64:65, :])
            nc.vector.reciprocal(rec[:, 1024:2048], upB[64:65, :])
        else:
            # reciprocal_approx_fast misreads PSUM operands on HW: stage
            # the denominator rows through SBUF first
            den = p.stat.tile([1, 2048], f32, tag="den", bufs=1)
            nc.vector.tensor_copy(den[:, 0:1024], upA[64:65, :])
            nc.vector.tensor_copy(den[:, 1024:2048], upB[64:65, :])
            nc.vector.reciprocal_approx_fast(rec[:], den[:])
        rbs = p.stat.tile([64, 2048], f32, tag="rbs", bufs=1)
        nc.gpsimd.partition_broadcast(rbs[:], rec[:])
        conc = p.act.tile([128, NFT * 256], bf16, tag="conc", bufs=2)
        for h in range(H):
            grp, hh = h // 4, h % 4
            nc.vector.tensor_mul(
                conc[(h % 2) * 64:(h % 2) * 64 + 64,
                     (h // 2) * 256:(h // 2) * 256 + 256],
                up[grp][0:64, hh * 256:(hh + 1) * 256],
                rbs[:, grp * 1024 + hh * 256: grp * 1024 + (hh + 1) * 256])

        w2a = p.wff.tile([128, 4096], bf16, tag="wff")
        nc.sync.dma_start(
            w2a[:].rearrange("p (k n) -> p k n", k=8),
            io["w2"][l][0:1024, :].rearrange("(k p) n -> p k n", p=128)[:])
        w2b = p.wff.tile([128, 4096], bf16, tag="wff")
        nc.sync.dma_start(
            w2b[:].rearrange("p (k n) -> p k n", k=8),
            io["w2"][l][1024:2048, :].rearrange("(k p) n -> p k n", p=128)[:])

        # ---- mha^T + residual + LN1 ----
        psW = pstile(f"psW{l}")
        for mf in range(NFT):
            for kf in range(NFT):
                nc.tensor.matmul(
                    psW[:, mf * 256:(mf + 1) * 256],
                    wo_t[:, kf * 512 + mf * 128: kf * 512 + (mf + 1) * 128],
                    conc[:, kf * 256:(kf + 1) * 256],
                    start=(kf == 0), stop=(kf == NFT - 1))
        res1 = p.act.tile([128, NFT * 256], f32, tag="res", bufs=2)
        for mf in range(NFT):
            nc.vector.tensor_scalar_add(
                res1[:, mf * 256:(mf + 1) * 256],
                psW[:, mf * 256:(mf + 1) * 256],
                bo_t[:, l * NFT + mf: l * NFT + mf + 1])
        nc.vector.tensor_add(res1[:], res1[:], ht[:])

        if STAGE == 4:
            bail(conc)
            return

        ln1f, ln1b = _layernorm(nc, p, ones_f, ones_r, res1,
                                g1_t, be1_t, l, "ln1", pstile)

        if STAGE == 5:
            bail(ln1b)
            return

        # ---- FFN ----
        a1t = []
        for ag in range(2):
            a1 = p.act.tile([128, 8 * 256], bf16, tag="a1", bufs=2)
            a1t.append(a1)
            for half in range(2):
                psA = pstile()
                for kf in range(NFT):
                    for m4 in range(4):
                        mt = ag * 8 + half * 4 + m4
                        wsrc = w1a if mt < 8 else w1b
                        moff = mt % 8
                        nc.tensor.matmul(
                            psA[:, m4 * 256:(m4 + 1) * 256],
                            wsrc[:, kf * 1024 + moff * 128:
                                 kf * 1024 + (moff + 1) * 128],
                            ln1b[:, kf * 256:(kf + 1) * 256],
                            start=(kf == 0 and m4 % 2 == 0),
                            stop=(kf == NFT - 1 and m4 % 2 == 1),
                            skip_group_check=True)
                for m4 in range(4):
                    mt = ag * 8 + half * 4 + m4
                    dst = a1[:, (half * 4 + m4) * 256:(half * 4 + m4 + 1) * 256]
                    src = psA[:, m4 * 256:(m4 + 1) * 256]
                    bia = b1_t[:, l * NMT + mt: l * NMT + mt + 1]
                    if m4 % 2 == 0:
                        nc.vector.tensor_scalar(
                            dst, src, bia, 0.0, ALU.add, ALU.max)
                    else:
                        nc.scalar.activation(dst, src, AF.Relu, bias=bia)

        psR = pstile(f"psR{l}")
        for kt2 in range(NMT):
            wsrc = w2a if kt2 < 8 else w2b
            koff = kt2 % 8
            for mf in range(NFT):
                nc.tensor.matmul(
                    psR[:, mf * 256:(mf + 1) * 256],
                    wsrc[:, koff * 512 + mf * 128:
                         koff * 512 + (mf + 1) * 128],
                    a1t[kt2 // 8][:, (kt2 % 8) * 256:(kt2 % 8 + 1) * 256],
                    start=(kt2 == 0 and mf % 2 == 0),
                    stop=(kt2 == NMT - 1 and mf % 2 == 1),
                    skip_group_check=True)
        res2 = p.act.tile([128, NFT * 256], f32, tag="res", bufs=2)
        for mf in range(NFT):
            nc.vector.tensor_scalar_add(
                res2[:, mf * 256:(mf + 1) * 256],
                psR[:, mf * 256:(mf + 1) * 256],
                b2_t[:, l * NFT + mf: l * NFT + mf + 1])
        nc.vector.tensor_add(res2[:], res2[:], ln1f[:])

        ht, htb = _layernorm(nc, p, ones_f, ones_r, res2,
                             g2_t, be2_t, l, "ln2", pstile)

    if SKIP_VOCAB:
        nc.sync.dma_start(io["out"][0][:, 0:NFT * TO], htb[:])
        return
    if NLAYERS < 2:
        for kf in range(NFT):
            nc.sync.dma_start(
                wout_sb[:, kf * VSP:(kf + 1) * VSP], io["wout"][kf])

    # ================= vocab-sharded projection =================
    # AllGather the final hidden state (bf16, feature-major) across all 8
    # cores, then project all 2048 tokens against this core's 4096-padded
    # vocab shard with Wout already resident in SBUF.
    contribH = p.dram.tile([E * TO], bf16, tag="contribH")
    nc.sync.dma_start(
        contribH[:].rearrange("(m p c) -> p m c", p=128, m=NFT),
        htb[:].rearrange("p (m c) -> p m c", m=NFT))
    gathH = p.dram.tile([NC, E * TO], bf16, tag="gathH",
                        addr_space="Local" if COLLFREE else "Shared")
    if COLLFREE:
        for rr in range(NC):
            nc.sync.dma_start(gathH[rr], contribH[:])
    else:
        nc.gpsimd.collective_compute(
            "AllGather", ALU.bypass,
            replica_groups=[[0, 1, 2, 3, 4, 5, 6, 7]],
            ins=[contribH[:]], outs=[gathH[:]])
    def project(tb_out, stat_fn):
        """project one 128-token block against the full vocab shard"""
        duos = [pstile() for _ in range(4)]
        for kf in range(NFT):
            for vc in range(VSP // 512):
                nc.tensor.matmul(
                    duos[vc // 2][:, (vc % 2) * 512:(vc % 2 + 1) * 512],
                    stat_fn(kf),
                    wout_sb[:, kf * VSP + vc * 512: kf * VSP + (vc + 1) * 512],
                    start=(kf == 0), stop=(kf == NFT - 1))
        for half in range(2):
            ob = p.out.tile([128, VSP // 2], bf16, tag="ob")
            for v2 in range(4):
                vc = half * 4 + v2
                dst = ob[:, v2 * 512:(v2 + 1) * 512]
                src = duos[vc // 2][:, (vc % 2) * 512:(vc % 2 + 1) * 512]
                if vc % 2 == 0:
                    nc.vector.tensor_copy(dst, src)
                else:
                    nc.scalar.copy(dst, src)
            nc.sync.dma_start(
                io["out"][tb_out][:, half * (VSP // 2):(half + 1) * (VSP // 2)],
                ob[:])

    # own token blocks first, straight from local htb -- overlaps the
    # AllGather; the host uses slots NTB..NTB+1 for this core's rows
    for h2 in range(NTT):
        project(NTB + h2,
                lambda kf, h2=h2: htb[:, kf * 256 + h2 * 128:
                                      kf * 256 + h2 * 128 + 128])

    htg = []
    hgv = gathH.rearrange("r (m p c) -> m p r c", m=NFT, p=128)
    for kf in range(NFT):
        t = p.hg.tile([128, NC * TO], bf16, tag="hg", name=f"htg{kf}")
        nc.sync.dma_start(t[:].rearrange("p (r c) -> p r c", r=NC), hgv[kf])
        htg.append(t)

    for tb in range(NTB):
        project(tb, lambda kf, tb=tb: htg[kf][:, tb * 128:(tb + 1) * 128])


def _layernorm(nc, p, ones_f, ones_r, res, g_t, b_t, l, name, pstile,
               out_dt=bf16, out_scale=1.0):
    """Feature-major layernorm over a [128, NFT*TO] f32 quad ->
    (f32, out_dt*out_scale).

    Partition sums stream the f32 residual bitcast to float32r (full rate
    at N>=256); rstd/mu are broadcast across partitions on the idle GpSimd
    engine instead of via PE matmuls. LN1/LN2 share lnf/lnb tags."""
    resb = p.act.tile([128, NFT * 256], bf16, tag="resb", bufs=2)
    nc.vector.tensor_copy(resb[:], res[:])
    sq = p.act.tile([128, NFT * 256], bf16, tag="sq", bufs=2)
    nc.vector.tensor_mul(sq[:], resb[:], resb[:])
    stats = pstile(f"stats_{name}{l}")
    for kf in range(NFT):
        nc.tensor.matmul(stats[0:1, 0:256], p.ones_fb[:, :],
                         resb[:, kf * 256:(kf + 1) * 256],
                         start=(kf == 0), stop=(kf == NFT - 1))
    for kf in range(NFT):
        nc.tensor.matmul(stats[32:33, 0:256], p.ones_fb[:, :],
                         sq[:, kf * 256:(kf + 1) * 256],
                         start=(kf == 0), stop=(kf == NFT - 1))
    sv = p.stat.tile([1, 4 * TO], f32, tag="stat", bufs=1)
    mu = sv[:, 0:TO]
    musq = sv[:, TO:2 * TO]
    var = sv[:, 2 * TO:3 * TO]
    std = sv[:, TO:2 * TO]          # reuses musq slot (musq dead)
    rstd = sv[:, 3 * TO:4 * TO]
    murstd = sv[:, 2 * TO:3 * TO]   # reuses var slot (var dead)
    nc.vector.tensor_scalar_mul(mu, stats[0:1, 0:256], 1.0 / E)
    nc.vector.tensor_mul(musq, mu, mu)
    nc.vector.tensor_scalar(var, stats[32:33, 0:256], 1.0 / E, EPS,
                            ALU.mult, ALU.add)
    nc.vector.tensor_sub(var, var, musq)
    # rstd = exp(-0.5*ln(var)); ln+exp live in the single pinned table set
    nc.scalar.activation(std, var, AF.Ln)
    nc.scalar.activation(rstd, std, AF.Exp, scale=-0.5)
    nc.vector.tensor_mul(murstd, mu, rstd)
    # broadcast rstd/murstd across partitions on GpSimd (PE stays free)
    rbmb = p.stat.tile([128, 512], f32, tag="rbmb", bufs=1)
    rb = rbmb[:, 0:256]
    mb = rbmb[:, 256:512]
    nc.gpsimd.partition_broadcast(rb, rstd)
    nc.gpsimd.partition_broadcast(mb, murstd)
    outf = p.act.tile([128, NFT * 256], f32, tag="lnf", bufs=3,
                      name=f"{name}f{l}")
    outb = p.act.tile([128, NFT * 256], out_dt, tag="lnb", bufs=3,
                      name=f"{name}b{l}")
    for kf in range(NFT):
        t = outf[:, kf * 256:(kf + 1) * 256]
        nc.vector.tensor_mul(t, res[:, kf * 256:(kf + 1) * 256], rb)
        nc.vector.tensor_sub(t, t, mb)
        nc.vector.tensor_scalar(
            t, t,
            g_t[:, l * NFT + kf: l * NFT + kf + 1],
            b_t[:, l * NFT + kf: l * NFT + kf + 1],
            ALU.mult, ALU.add)
        # per-chunk cast so downstream matmuls start on chunk 0 while
        # later chunks are still being normalized
        if out_scale == 1.0:
            nc.vector.tensor_copy(outb[:, kf * 256:(kf + 1) * 256], t)
        else:
            nc.vector.tensor_scalar_mul(
                outb[:, kf * 256:(kf + 1) * 256], t, out_scale)
    return outf, outb


def _prep_inputs(x, tok_emb, pos_emb, Wq, bq, Wk, bk, Wv, bv, Wo, bo,
                 W1, b1, W2, b2, ln1_g, ln1_b, ln2_g, ln2_b, Wout, bout):
    """Host-side sharding: returns in_maps for the 8 cores."""
    x = np.asarray(x)
    h0 = np.asarray(tok_emb)[x] + np.asarray(pos_emb)[None, :, :]   # [B,S,E] f32
    h0t = np.ascontiguousarray(np.transpose(h0, (0, 2, 1)))          # [B,E,S]

    F8 = mybir.dt.np(mybir.dt.float8e4)
    scale = 1.0 / np.sqrt(D)
    wq_h = (np.transpose(np.asarray(Wq), (0, 2, 1, 3)).reshape(L, E, H * D)
            * scale).astype(BF16)
    wk_h = np.transpose(np.asarray(Wk), (0, 2, 1, 3)).reshape(L, E, H * D).astype(BF16)
    wv_h = np.transpose(np.asarray(Wv), (0, 2, 1, 3)).reshape(L, E, H * D).astype(BF16)
    wo_h = np.asarray(Wo).astype(BF16)
    w1_h = np.asarray(W1).astype(BF16)
    w2_h = np.asarray(W2).astype(BF16)
    bq_h = (np.asarray(bq).reshape(L, H * D) * scale).astype(np.float32)
    bk_h = np.asarray(bk).reshape(L, H * D).astype(np.float32)
    bv_c = np.asarray(bv).reshape(L, H * D).astype(np.float32)
    bo_eff = (np.asarray(bo) + np.einsum("lc,lce->le", bv_c, np.asarray(Wo))
              ).astype(np.float32)
    wout_np = np.zeros((NFT, 128, NC, VSP), dtype=BF16)
    wfull = np.asarray(Wout).astype(BF16).reshape(NFT, 128, V)
    for c in range(NC):
        wout_np[:, :, c, :VS] = wfull[:, :, c * VS:(c + 1) * VS]

    # pack all [L, E]-style bias/scale vectors (+ the per-core key-block
    # mask appended later) into one [128, 200] f32 tensor: col l*n+k holds
    # arr[l, k*128+p] for partition p
    def pk(arr, n=NFT):
        return np.ascontiguousarray(
            np.asarray(arr, dtype=np.float32)
            .reshape(L, n, 128).transpose(2, 0, 1).reshape(128, L * n))

    bvec_c = np.concatenate([
        pk(bq_h), pk(bk_h), pk(bo_eff), pk(b2),
        pk(ln1_g), pk(ln1_b), pk(ln2_g), pk(ln2_b),
        pk(b1, n=NMT)], axis=1)

    common = dict(
        wq=wq_h, wk=wk_h, wv=wv_h, wo=wo_h, w1=w1_h, w2=w2_h,
    )

    in_maps = []
    for c in range(NC):
        b, j = c // G, c % G
        # own-block causal mask, replicated across 4 heads (both head
        # groups reuse the same tile): [NTT*128, 4*TO]
        qpos = j * TO + np.arange(TO)[None, :]
        moR = np.zeros((NTT * 128, 4 * TO), dtype=BF16)
        for lt in range(NTT):
            kpos = j * TO + lt * 128 + np.arange(128)[:, None]
            m = (kpos <= qpos).astype(BF16)          # [128, TO]
            moR[lt * 128:(lt + 1) * 128] = np.tile(m, (1, 4))
        # gathered-path visibility per 128-key block: fully visible (0.0)
        # only strictly below this core's own rows; own rows come from the
        # local pass, everything else exp(-30)-masked
        maskb = np.full((128, NKT), -30.0, np.float32)
        maskb[:, :2 * j] = 0.0
        in_maps.append(dict(
            common,
            h0t_full=h0t[b].astype(BF16),
            h0t_own=np.ascontiguousarray(
                h0t[b][:, j * TO:(j + 1) * TO]).astype(np.float32),
            bvec=np.ascontiguousarray(
                np.concatenate([bvec_c, maskb], axis=1)),
            maskOwnR=moR,
            wout=np.ascontiguousarray(wout_np[:, :, c, :]),
        ))
    return in_maps


def _finish_output(res, bout):
    bout = np.asarray(bout, dtype=np.float32)
    logits = np.empty((B, S, V), dtype=np.float32)
    for c in range(NC):
        o = np.asarray(res.results[c]["out"], dtype=np.float32)  # [NTB+2,...]
        for tb in range(NTB):
            r = tb // 2
            bb, j = r // G, r % G
            t0 = j * TO + (tb % 2) * 128
            # own token blocks come from the early (pre-AllGather) slots
            src = NTB + (tb % 2) if r == c else tb
            logits[bb, t0:t0 + 128, c * VS:(c + 1) * VS] = o[src][:, :VS]
    logits += bout[None, None, :]
    return logits


def kernel(**inputs):
    if "nc" not in _cache:
        _cache["nc"] = build_nc()
    nc = _cache["nc"]
    inputs = {k: np.asarray(v) for k, v in inputs.items()}
    in_maps = _prep_inputs(**inputs)
    res = run_bass_kernel_spmd(nc, in_maps, list(range(NC)))
    return _finish_output(res, inputs["bout"])
